# revision 1
# baseline (speedup 1.0000x reference)
"""Causal multi-head attention (B=2, T=2048, D=1024, H=16) on 8 TRN2 NeuronCores.

Sharding: core c = (batch b = c//4, head-group g = c%4). Each core owns 4 heads
(= 256 contiguous dims of D) of one batch: Megatron-style tensor parallelism on
heads x data parallelism on batch. ~238-255us vs the 352us fp32-ReduceScatter
baseline this evolved from.

Design (single fully-pipelined phase; everything but span 0's attention is
emitted as "fillers" interleaved into the attention kt loops so the in-order
PE queue has no phase boundaries, no lumps, and never head-of-line blocks):
  - Out-projection reduction via per-q-span 8-way bf16 AllToAll of the
    normalized attention output yT (rank r's territory = 64-col q-block r of
    each 512-q span, for BOTH batches -- SPMD-uniform, no junk shards). Each
    core then out-projects its territory with the full Wo: no partial sums,
    no fp32 reduce; ~8x less wire than a ReduceScatter of fp32 partials.
  - Score matmuls pack the two heads of an mc-chunk as two concurrent K=64
    row-group tiles ((0,0)/(64,0)) into one [128, 2*512] 2-bank PSUM tile;
    ONE exp activation per pair tile ([128,1024] = 1.15us vs 2x 0.72us).
    The kt loop software-pipelines scores kt+1 ahead of the AV matmuls of kt
    so the ACT exp stream saturates. ACT runs only exp + the 1/den chains
    (exp(-ln(den)), same table set) + out-projection PSUM evacuations.
  - AV uses the v_aug 65th-column trick (denominator accumulates as row 64);
    normalization = PE rank-1 broadcast of 1/den + in-place DVE multiply,
    one span behind attention. Diagonal-tile score/AV matmuls stream only
    the causally-valid q sub-range; the only mask ever applied is a single
    [128,128] tril block (identical for every diagonal tile).
  - Projections (q/k/v for span s+1) are fillers in span s's kt loop; a PE
    warmup matmul stream covers the initial x-load window (HAM un-throttle).
  - Out-projection batches 2 spans (M=128); pair 0 brackets span 3's
    normalize tail, and span 3's A2A is split into mc halves so only the
    odd-kc matmuls of pair 1 wait on the final 128KB half-exchange.

Dtypes: all matmul operands bf16 with fp32 PSUM accumulation; softmax exp(s)
without row-max (scores O(1), scale folded into Wq host-side); bo folded into
the out-projection as a rank-1 seed matmul. End-to-end rel err ~5.9e-3.
"""

import os
import numpy as np
import ml_dtypes

BF16 = ml_dtypes.bfloat16

B, T, D, H = 2, 2048, 1024, 16
HD = D // H                     # 64
NCORES = 8
GROUPS = 4                      # cores per batch (tensor-parallel degree)
HL = H // GROUPS                # heads per core = 4
DL = D // GROUPS                # dims per core = 256
SP = 512                        # free-dim span per matmul (one PSUM bank, fp32)
QS = T // SP                    # 4 q spans
KT = T // 128                   # 16 k tiles
QB = 64                         # q columns per rank territory per span
SCALE = HD ** -0.5

_CACHE = {}


def _build_program():
    import concourse.bass as bass  # noqa: F401  (registers bass machinery)
    import concourse.tile as tile
    from concourse import bacc, mybir

    f32 = mybir.dt.float32
    bf16 = mybir.dt.bfloat16
    Exp = mybir.ActivationFunctionType.Exp
    Ln = mybir.ActivationFunctionType.Ln

    nc = bacc.Bacc("TRN2", target_bir_lowering=False, debug=False,
                   num_devices=NCORES)

    xT = nc.dram_tensor("xT", [D, T], bf16, kind="ExternalInput")
    wqT = nc.dram_tensor("wqT", [D, DL], bf16, kind="ExternalInput")
    wkT = nc.dram_tensor("wkT", [D, DL], bf16, kind="ExternalInput")
    wvT = nc.dram_tensor("wvT", [D, DL], bf16, kind="ExternalInput")
    woT = nc.dram_tensor("woT", [D, D], bf16, kind="ExternalInput")
    bqP = nc.dram_tensor("bqP", [128, 2], f32, kind="ExternalInput")
    bkP = nc.dram_tensor("bkP", [128, 2], f32, kind="ExternalInput")
    bv = nc.dram_tensor("bv", [1, DL], bf16, kind="ExternalInput")
    bo = nc.dram_tensor("bo", [1, D], bf16, kind="ExternalInput")
    maskd = nc.dram_tensor("maskd", [128, 128], bf16, kind="ExternalInput")
    onesb = nc.dram_tensor("onesb", [1, 128], bf16, kind="ExternalInput")
    out_ext = nc.dram_tensor("out", [QS, 128, D], f32, kind="ExternalOutput")

    ALL8 = [[0, 1, 2, 3, 4, 5, 6, 7]]

    with tile.TileContext(nc) as tc:
        with tc.tile_pool(name="main", bufs=1) as main, \
             tc.tile_pool(name="dram", bufs=1, space="DRAM") as dram:
            qT_s = main.tile([128, 2, T], bf16)
            kT_s = main.tile([128, 2, T], bf16)
            v_s = main.tile([128, KT, HL * 65], bf16)
            yT_s = main.tile([128, 2, T], bf16)
            woT_s = main.tile([128, 8, D], bf16)
            bq_s = main.tile([128, 2], f32)
            bk_s = main.tile([128, 2], f32)
            bv_s = main.tile([1, DL], bf16)
            bo_s = main.tile([1, D], bf16)
            onesb_s = main.tile([128, 128], bf16)
            bv_bc = main.tile([128, DL], bf16)
            maskd_s = main.tile([128, 128], bf16)
            warm_s = main.tile([128, 2], f32)
            warm_sb = main.tile([128, SP], bf16)

            # per-span A2A staging (separate tiles avoid false DRAM deps)
            a2a_in = [dram.tile([8, DL, QB], bf16, name=f"a2ain{i}")
                      for i in range(QS)]
            a2a_out = [dram.tile([8, DL, QB], bf16, name=f"a2aout{i}")
                       for i in range(QS)]
            # span 3 exchanges in mc halves (contiguous per-half buffers)
            a2a_in3 = [dram.tile([8, 128, QB], bf16, name=f"a2ain3h{m}")
                       for m in range(2)]
            a2a_out3 = [dram.tile([8, 128, QB], bf16, name=f"a2aout3h{m}")
                        for m in range(2)]

            # PE warmup: ~15us of back-to-back matmuls on scratch data while
            # the input DMAs stream in, so HAM un-throttles (K=4/8 -> 8/8)
            # before the first real projection matmul
            nc.vector.memset(warm_sb, 1.0)
            with tc.tile_pool(name="warm_psum", bufs=1,
                              space="PSUM") as warm_psum:
                wps = warm_psum.tile([128, SP], f32, tag="w")
                for _ in range(32):
                    nc.tensor.matmul(wps, lhsT=warm_sb[:, 0:128],
                                     rhs=warm_sb, start=True, stop=True)

            # tiny high-priority loads on the sync queue
            nc.sync.dma_start(out=bq_s, in_=bqP[:])
            nc.sync.dma_start(out=bk_s, in_=bkP[:])
            # pre-load the ACT Log+Exp table during phase-1 DMAs so span 0's
            # first real exp doesn't pay the ~2.7us table switch
            nc.scalar.activation(warm_s, warm_sb[:, 0:2], Ln)
            nc.scalar.activation(warm_s, warm_sb[:, 0:2], Exp)
            # small loads on the scalar queue
            nc.scalar.dma_start(out=onesb_s,
                                in_=onesb[:].to_broadcast([128, 128]))
            nc.scalar.dma_start(out=bv_bc, in_=bv[:].to_broadcast([128, DL]))
            nc.scalar.dma_start(out=bo_s, in_=bo[:])
            # ones column at index 64 of each head's 65-wide block of v_aug
            nc.vector.memset(v_s, 1.0)

            # ---------------- input loads ----------------
            xt_s = main.tile([128, 8, T], bf16)
            wq_s = main.tile([128, 8, DL], bf16)
            wk_s = main.tile([128, 8, DL], bf16)
            wv_s = main.tile([128, 8, DL], bf16)

            # critical path first, in kc consumption order: the first qT
            # matmul needs wq[0] + x[0]; interleave so chunk kc lands
            # roughly in order (x odd chunks on the gpsimd queue)
            wq_r = wqT[:].rearrange("(c p) n -> c p n", p=128)
            xT_r = xT[:].rearrange("(c p) t -> c p t", p=128)
            xq = [nc.sync, nc.gpsimd, nc.scalar]
            for c in range(8):
                nc.sync.dma_start(out=wq_s[:, c, :], in_=wq_r[c])
                xq[c % 3].dma_start(out=xt_s[:, c, :], in_=xT_r[c])
            # wk/wv on the scalar queue (needed by the prologue kT/v groups),
            # followed by the attention/outproj bulk loads
            for w_s, w_d in ((wk_s, wkT), (wv_s, wvT)):
                w_r = w_d[:].rearrange("(c p) n -> c p n", p=128)
                for c in range(8):
                    nc.scalar.dma_start(out=w_s[:, c, :], in_=w_r[c])
            nc.scalar.dma_start(out=maskd_s, in_=maskd[:])
            woT_r = woT[:].rearrange("(c p) n -> c p n", p=128)
            for c in range(8):
                nc.scalar.dma_start(out=woT_s[:, c, :], in_=woT_r[c])

            # ---- single pipelined phase: projections for span s+1, the
            # normalize+A2A of span s-1 and the out-projection (2-span
            # batches) are all emitted as "fillers" interleaved into span
            # s's attention kt loop, so ACT starts its exp stream as soon as
            # span 0's inputs exist and the in-order PE queue never has
            # span-boundary lumps or head-of-line A2A blocks
            with tc.tile_pool(name="attn_t", bufs=4) as attn_t, \
                 tc.tile_pool(name="nrm", bufs=2) as nrm, \
                 tc.tile_pool(name="op_sb", bufs=4) as op_sb, \
                 tc.tile_pool(name="sc_psum", bufs=2, space="PSUM") as sc_psum, \
                 tc.tile_pool(name="av_psum", bufs=2, space="PSUM") as av_psum, \
                 tc.tile_pool(name="op_psum", bufs=2, space="PSUM") as op_psum:

                def attention_span(qs, fillers, self_norm=False):
                    # denominator rows at partitions 0/32/64/96 (engine APs
                    # must start 32-aligned); memset keeps unused rows finite
                    den_stack = nrm.tile([97, SP], f32, tag="den")
                    nc.vector.memset(den_stack, 1.0)
                    # 1/den as exp(-log(den)) on ACT, one chain per head pair
                    # (Log+Exp share a table set; DVE's iterative reciprocal
                    # would cost 3.4us per span)
                    lg = nrm.tile([97, SP], f32, tag="lg")
                    rec_bf = nrm.tile([97, SP], bf16, tag="recf")
                    nkt = 4 * qs + 4  # causal: later k tiles are all-masked
                    span = slice(qs * SP, (qs + 1) * SP)
                    steps_left = 2 * nkt

                    for p in range(2):  # head pair = mc chunk p
                        qa = qT_s[0:64, p, span]
                        qb = qT_s[64:128, p, span]
                        ya = av_psum.tile([65, SP], f32, tag="av")
                        yb = av_psum.tile([65, SP], f32, tag="av")

                        def sc_pair(kt):
                            # diagonal tiles: q columns < 128*(kt-4qs) are
                            # fully masked; stream only the valid sub-range
                            # (stale left-cols are never read -- AV slices
                            # identically)
                            j0 = max(0, (kt - 4 * qs) * 128)
                            scp = sc_psum.tile([128, 2 * SP], f32, tag="sc")
                            nc.tensor.matmul(
                                scp[:, j0:SP],
                                lhsT=kT_s[0:64, p, kt * 128:(kt + 1) * 128],
                                rhs=qa[:, j0:SP], start=True, stop=True)
                            nc.tensor.matmul(
                                scp[:, SP + j0:2 * SP],
                                lhsT=kT_s[64:128, p, kt * 128:(kt + 1) * 128],
                                rhs=qb[:, j0:SP], start=True, stop=True)
                            return scp

                        # software pipeline: scores kt+1 are emitted before
                        # the AV matmuls of kt so the in-order PE queue keeps
                        # feeding ACT while AV waits on exp kt
                        scp = sc_pair(0)
                        for kt in range(nkt):
                            atp = attn_t.tile([128, 2 * SP], bf16, tag="at")
                            j0e = max(0, (kt - 4 * qs) * 128)
                            if j0e >= 256:
                                # mostly-masked diagonal tile: two narrow
                                # exps over the valid ranges beat one full-
                                # width one (2*(512-j0+352) < 1024+352 cyc)
                                nc.scalar.activation(atp[:, j0e:SP],
                                                     scp[:, j0e:SP], Exp)
                                nc.scalar.activation(
                                    atp[:, SP + j0e:2 * SP],
                                    scp[:, SP + j0e:2 * SP], Exp)
                            else:
                                nc.scalar.activation(atp, scp, Exp)
                            if kt >= 4 * qs:
                                # diagonal tile: only its 128-col diagonal
                                # block needs masking (left cols are never
                                # read, right cols are fully valid) and that
                                # block is the same tril(128) for every tile
                                jm = (kt - 4 * qs) * 128
                                nc.vector.tensor_mul(
                                    atp[:, jm:jm + 128],
                                    atp[:, jm:jm + 128], maskd_s)
                                nc.vector.tensor_mul(
                                    atp[:, SP + jm:SP + jm + 128],
                                    atp[:, SP + jm:SP + jm + 128], maskd_s)
                            if kt + 1 < nkt:
                                scp = sc_pair(kt + 1)
                            j0 = max(0, (kt - 4 * qs) * 128)
                            nc.tensor.matmul(
                                ya[:, j0:SP],
                                lhsT=v_s[:, kt, (2 * p) * 65:
                                             (2 * p + 1) * 65],
                                rhs=atp[:, j0:SP],
                                start=(kt == 0), stop=(kt == nkt - 1))
                            nc.tensor.matmul(
                                yb[:, j0:SP],
                                lhsT=v_s[:, kt, (2 * p + 1) * 65:
                                             (2 * p + 2) * 65],
                                rhs=atp[:, SP + j0:2 * SP],
                                start=(kt == 0), stop=(kt == nkt - 1))
                            # spread pending post-processing of earlier spans
                            # evenly across this span's kt steps
                            n_pop = -(-len(fillers) // steps_left)
                            for _ in range(n_pop):
                                fillers.pop(0)()
                            steps_left -= 1
                        # evacuate unnormalized yT' + denominators on DVE so
                        # the PSUM banks free for the next pair
                        nc.vector.tensor_copy(yT_s[0:64, p, span], ya[0:64, :])
                        nc.vector.tensor_copy(yT_s[64:128, p, span],
                                              yb[0:64, :])
                        nc.vector.tensor_copy(
                            den_stack[64 * p:64 * p + 1, :], ya[64:65, :])
                        nc.vector.tensor_copy(
                            den_stack[64 * p + 32:64 * p + 33, :],
                            yb[64:65, :])
                        if self_norm:
                            # last span: per-pair reciprocal chain; pair 0's
                            # rows normalize during pair 1's sweep so only
                            # pair 1's chain remains after the last matmul
                            sl = slice(64 * p, 64 * p + 64 if p == 0 else 97)
                            nc.scalar.activation(lg[sl, :], den_stack[sl, :],
                                                 Ln)
                            nc.scalar.activation(rec_bf[sl, :], lg[sl, :],
                                                 Exp, scale=-1.0)
                            if p == 0:
                                # front-insert: the mc0 normalize + half-A2A
                                # must fire early in pair 1's sweep, not
                                # behind the queued out-projection fillers
                                fillers[0:0] = [norm_h(qs, rec_bf, 0),
                                                norm_h(qs, rec_bf, 1),
                                                stage_a2a_mc(qs, 0)]
                    if not self_norm:
                        # one reciprocal chain at span end: it fills ACT's
                        # span-boundary gap instead of its saturated stream
                        nc.scalar.activation(lg, den_stack, Ln)
                        nc.scalar.activation(rec_bf, lg, Exp, scale=-1.0)
                    return rec_bf

                def norm_h(qs, rec_bf, h):
                    def f():
                        span = slice(qs * SP, (qs + 1) * SP)
                        mc, r0 = divmod(h, 2)
                        r0 *= 64
                        rb = op_psum.tile([64, SP], f32, tag="op")
                        r0p = 32 * h
                        nc.tensor.matmul(rb,
                                         lhsT=onesb_s[r0p:r0p + 1, 0:64],
                                         rhs=rec_bf[r0p:r0p + 1, :],
                                         start=True, stop=True,
                                         tile_position=(r0p, 0))
                        nc.vector.tensor_mul(yT_s[r0:r0 + 64, mc, span],
                                             yT_s[r0:r0 + 64, mc, span],
                                             rb)
                    return f

                def stage_a2a(qs):
                    def f():
                        span = slice(qs * SP, (qs + 1) * SP)
                        in_r = a2a_in[qs][:].rearrange(
                            "j (two p) q -> two p j q", p=128)
                        for mc in range(2):
                            nc.sync.dma_start(
                                out=in_r[mc],
                                in_=yT_s[:, mc, span].rearrange(
                                    "p (j q) -> p j q", q=QB))
                        nc.gpsimd.collective_compute(
                            "AllToAll", mybir.AluOpType.bypass,
                            replica_groups=ALL8,
                            ins=[a2a_in[qs][:].opt()],
                            outs=[a2a_out[qs][:].opt()])
                    return f

                def stage_a2a_mc(qs, mc):
                    # half exchange: shard rows = the mc dims-half only --
                    # lets span 3's mc0 half fly while pair 1 still computes
                    def f():
                        span = slice(qs * SP, (qs + 1) * SP)
                        nc.sync.dma_start(
                            out=a2a_in3[mc][:].rearrange("j p q -> p j q"),
                            in_=yT_s[:, mc, span].rearrange(
                                "p (j q) -> p j q", q=QB))
                        nc.gpsimd.collective_compute(
                            "AllToAll", mybir.AluOpType.bypass,
                            replica_groups=ALL8,
                            ins=[a2a_in3[mc][:].opt()],
                            outs=[a2a_out3[mc][:].opt()])
                    return f

                def norm_a2a_fillers(qs, rec_bf):
                    return ([norm_h(qs, rec_bf, h) for h in range(HL)]
                            + [stage_a2a(qs)])

                def proj_qk(w_s, b_s, dst, mc, s):
                    # qT/kT group: out[dims-chunk mc, t-span s]; bias added
                    # on DVE during the PSUM->SBUF evacuation
                    def f():
                        ps = op_psum.tile([128, SP], f32, tag="op",
                                          name="pj")
                        for kc in range(8):
                            nc.tensor.matmul(
                                ps,
                                lhsT=w_s[:, kc, mc * 128:(mc + 1) * 128],
                                rhs=xt_s[:, kc, s * SP:(s + 1) * SP],
                                start=(kc == 0), stop=(kc == 7))
                        nc.vector.tensor_scalar_add(
                            dst[:, mc, s * SP:(s + 1) * SP],
                            ps, b_s[:, mc:mc + 1])
                    return f

                def proj_v(mt):
                    # v tile in natural [t, d] layout; bias via DVE add into
                    # the 65-stride v_aug slots
                    def f():
                        ps = op_psum.tile([128, SP], f32, tag="op",
                                          name="pjv")
                        for kc in range(8):
                            nc.tensor.matmul(
                                ps[:, 0:DL],
                                lhsT=xt_s[:, kc, mt * 128:(mt + 1) * 128],
                                rhs=wv_s[:, kc, :],
                                start=(kc == 0), stop=(kc == 7))
                        nc.vector.tensor_add(
                            v_s[:, mt, :].rearrange(
                                "p (h d) -> p h d", d=65)[:, :, 0:64],
                            ps[:, 0:DL].rearrange("p (h d) -> p h d", d=64),
                            bv_bc.rearrange("p (h d) -> p h d", d=64))
                    return f

                def proj_span_fillers(s):
                    fs = []
                    for mc in range(2):
                        fs.append(proj_qk(wq_s, bq_s, qT_s, mc, s))
                    for mc in range(2):
                        fs.append(proj_qk(wk_s, bk_s, kT_s, mc, s))
                    for mt in range(4 * s, 4 * s + 4):
                        fs.append(proj_v(mt))
                    return fs

                def outproj_fillers(pair):
                    """Out-projection for spans (2*pair, 2*pair+1) with
                    M=128 (both spans' 64-q territories stacked), split into
                    per-(b,ns) filler chunks. For pair 1, span 3's A2A is
                    exchanged in mc halves: kc chunks with kc%2==0 depend
                    only on the early half, so they are computed first and
                    only the odd-kc matmuls wait on the final half-A2A."""
                    yg = op_sb.tile([128, 8, 2, 2 * QB], bf16, tag="yg")

                    def gather(sp):
                        def f():
                            out_r = a2a_out[sp][:].rearrange(
                                "(b j2) (h p) q -> b p (j2 h) q", j2=4, p=128)
                            m = sp % 2
                            for b in range(2):
                                nc.sync.dma_start(
                                    out=yg[:, :, b, m * QB:(m + 1) * QB],
                                    in_=out_r[b])
                        return f

                    def gather_h(sp, h):
                        # only the kc chunks with kc%2 == h (one half-A2A)
                        def f():
                            out_r = a2a_out3[h][:].rearrange(
                                "(b j2) p q -> b p j2 q", j2=4)
                            m = sp % 2
                            for b in range(2):
                                nc.sync.dma_start(
                                    out=yg[:, h::2, b,
                                           m * QB:(m + 1) * QB],
                                    in_=out_r[b])
                        return f

                    def po_seed(b, ns, po_box, pool, ptag):
                        def f():
                            po_box.append(
                                pool.tile([128, SP], f32, tag=ptag,
                                          name="po"))
                            # rank-1 seed adds bo along the free dim; the
                            # only start=True matmul into this bank
                            nc.tensor.matmul(
                                po_box[0][:, 0:SP], lhsT=onesb_s[0:1, :],
                                rhs=bo_s[0:1, ns * SP:(ns + 1) * SP],
                                start=True, stop=False)
                        return f

                    def po_mms(b, ns, kcs, po_box, last=False):
                        def f():
                            po = po_box[0]
                            for kc in kcs:
                                nc.tensor.matmul(
                                    po[:, 0:SP], lhsT=yg[:, kc, b, :],
                                    rhs=woT_s[:, kc, ns * SP:(ns + 1) * SP],
                                    start=False, stop=(last and kc == kcs[-1]))
                            if last:
                                # evacuate; rows 0:64 are span 2*pair,
                                # 64:128 span 2*pair+1. Alternate ACT/DVE so
                                # the final four evacuations run in parallel
                                ob = op_sb.tile([128, SP], f32, tag="ob")
                                if (b + ns) % 2 == 0:
                                    nc.scalar.copy(ob, po[:, 0:SP])
                                else:
                                    nc.vector.tensor_copy(ob, po[:, 0:SP])
                                for m in range(2):
                                    nc.sync.dma_start(
                                        out=out_ext[2 * pair + m,
                                                    b * 64:(b + 1) * 64,
                                                    ns * SP:(ns + 1) * SP],
                                        in_=ob[m * 64:(m + 1) * 64, :])
                        return f

                    if pair == 0:
                        fs = [gather(0), gather(1)]
                        for b in range(2):
                            for ns in range(2):
                                box = []
                                # b=1 pos run in the den-chain/A2A(3) bridge:
                                # the sc ring is idle there, so they don't
                                # serialize behind the op ring's evacuations
                                pool, ptag = ((op_psum, "op") if b == 0
                                              else (sc_psum, "sc"))
                                fs.append(po_seed(b, ns, box, pool, ptag))
                                fs.append(po_mms(b, ns, [0, 1, 2], box))
                                fs.append(po_mms(b, ns, [3, 4, 5], box))
                                fs.append(po_mms(b, ns, [6, 7], box,
                                                 last=True))
                        return fs
                    # pair 1: 4 concurrent accumulators (2 borrow the idle
                    # sc ring), even kc first, odd kc after the half-A2A
                    fs = [gather(2), gather_h(3, 0)]
                    boxes = {}
                    for i, (b, ns) in enumerate(((0, 0), (0, 1),
                                                 (1, 0), (1, 1))):
                        boxes[(b, ns)] = []
                        pool, ptag = ((op_psum, "op") if i < 2
                                      else (sc_psum, "sc"))
                        fs.append(po_seed(b, ns, boxes[(b, ns)], pool, ptag))
                        fs.append(po_mms(b, ns, [0, 2, 4, 6],
                                         boxes[(b, ns)]))
                    fs.append(gather_h(3, 1))
                    for b in range(2):
                        for ns in range(2):
                            fs.append(po_mms(b, ns, [1, 3, 5, 7],
                                             boxes[(b, ns)], last=True))
                    return fs

                # prologue: span 0's own inputs, then fillers carry the rest
                for f in proj_span_fillers(0):
                    f()
                pend = []
                opf = None
                for qs in range(QS):
                    if qs < 3:
                        # span qs+1's projections must drain by end of span qs
                        pend.extend(proj_span_fillers(qs + 1))
                    else:
                        # A2A(0)/(1) completed spans ago: half of pair 0's
                        # out-projection fills span 3's ACT-bound idle
                        opf = outproj_fillers(0)
                        pend.extend(opf[:10])
                    rec_bf = attention_span(qs, pend, self_norm=(qs == 3))
                    if qs < 3:
                        pend.extend(norm_a2a_fillers(qs, rec_bf))
                for f in pend:
                    f()
                # pair-0 out-projection is split three ways: half rode span
                # 3's kt loop as fillers (PE density -> HAM warmth), one po
                # bridges the den chain, and the last one covers the A2A(3b)
                # flight; pair 1 follows its completion
                for f in opf[10:14]:
                    f()
                norm_h(3, rec_bf, 2)()
                norm_h(3, rec_bf, 3)()
                stage_a2a_mc(3, 1)()
                for f in opf[14:]:
                    f()
                for f in outproj_fillers(1):
                    f()

    nc.compile()
    return nc


def _get_program():
    if "nc" not in _CACHE:
        _CACHE["nc"] = _build_program()
    return _CACHE["nc"]


def _make_in_maps(x, mask, Wq, bq, Wk, bk, Wv, bv, Wo, bo):
    x = np.asarray(x, np.float32)
    mask = np.asarray(mask, bool)
    Wq = np.asarray(Wq, np.float32)
    Wk = np.asarray(Wk, np.float32)
    Wv = np.asarray(Wv, np.float32)
    Wo = np.asarray(Wo, np.float32)
    bq = np.asarray(bq, np.float32)
    bk = np.asarray(bk, np.float32)
    bv = np.asarray(bv, np.float32)
    bo = np.asarray(bo, np.float32)

    woT = np.ascontiguousarray(Wo.T).astype(BF16)
    in_maps = []
    per_batch = {}
    for b in range(B):
        xTb = np.ascontiguousarray(x[b].T)
        # the only masking the kernel applies is the 128x128 diagonal
        # block (identical for every diagonal tile of a causal mask)
        md = mask[b, 0].T[0:128, 0:128].astype(np.float32)
        per_batch[b] = (xTb, md)
    for c in range(NCORES):
        b, g = divmod(c, GROUPS)
        sl = slice(g * DL, (g + 1) * DL)
        xTb, md = per_batch[b]
        in_maps.append({
            "xT": xTb.astype(BF16),
            "wqT": np.ascontiguousarray((Wq[sl] * SCALE).T).astype(BF16),
            "wkT": np.ascontiguousarray(Wk[sl].T).astype(BF16),
            "wvT": np.ascontiguousarray(Wv[sl].T).astype(BF16),
            "woT": woT,
            "bqP": np.ascontiguousarray((bq[sl] * SCALE).reshape(2, 128).T),
            "bkP": np.ascontiguousarray(bk[sl].reshape(2, 128).T),
            "bv": bv[sl].reshape(1, DL).astype(BF16),
            "bo": bo.reshape(1, D).astype(BF16),
            "maskd": md.astype(BF16),
            "onesb": np.ones((1, 128), BF16),
        })
    return in_maps


def _capture_profile(nc, in_maps, tmpdir):
    """Run with NTFF capture and process the profile ourselves (the stock
    trace path can't handle the duplicate-executable NTFFs the axon relay
    produces). Returns (results, exec_time_ns|None)."""
    import glob
    import json
    import re
    import subprocess
    from trn_agent_boot.trn_boot import _ntff_profile_via_ctypes
    from concourse import bass2jax

    hook = _ntff_profile_via_ctypes("/opt/axon/libaxon_pjrt.so")
    if hook is None:
        raise RuntimeError("libaxon_pjrt.so lacks NTFF profile symbols")
    os.makedirs(tmpdir, exist_ok=True)
    with hook(tmpdir, [0]):
        results = bass2jax.run_bass_via_pjrt(nc, in_maps, n_cores=NCORES)

    # group NTFF/NEFF pairs by executable id; use the newest executable
    ntffs = glob.glob(os.path.join(tmpdir, "*_body*-device*.ntff"))
    best = None
    for f in ntffs:
        if re.search(r"executable(\d+)-device000000", f):
            if best is None or os.path.getmtime(f) > os.path.getmtime(best):
                best = f
    if best is None:
        raise RuntimeError(f"no NTFF produced in {tmpdir}")
    neff = re.sub(r"-device\d+-execution-\d+\.ntff$", ".neff", best)
    out_json = os.path.join(tmpdir, "prof.json")
    subprocess.check_call(
        ["neuron-profile", "view", "--ignore-nc-buf-usage", "-s", best,
         "-n", neff, "--output-format=json", f"--output-file={out_json}"],
        cwd=tmpdir)
    summary = json.load(open(out_json))["summary"][0]
    return results, int(summary["total_time"] * 1e9)


def kernel(x, mask, Wq, bq, Wk, bk, Wv, bv, Wo, bo):
    from concourse import bass_utils

    in_maps = _make_in_maps(x, mask, Wq, bq, Wk, bk, Wv, bv, Wo, bo)
    nc = _get_program()

    trace = bool(int(os.environ.get("MHA_TRACE", "0")))
    tmpdir = os.environ.get("MHA_TRACE_DIR") or None
    results = None
    if trace and tmpdir:
        try:
            results, exec_ns = _capture_profile(nc, in_maps, tmpdir)
            _CACHE["last_exec_time_ns"] = exec_ns
        except Exception as e:  # profiling is best-effort
            print(f"profiling unavailable: {type(e).__name__}: {e}")
            results = None
    if results is None:
        results = bass_utils.run_bass_kernel_spmd(
            nc, in_maps, core_ids=list(range(NCORES))).results
        _CACHE.setdefault("last_exec_time_ns", None)

    # core c's out[qs] holds rows (q = qs*512 + c*64 + i) for batch 0
    # (rows 0-63) and batch 1 (rows 64-127)
    out = np.empty((B, T, D), np.float32)
    for c in range(NCORES):
        o = results[c]["out"]
        for qs in range(QS):
            q0 = qs * SP + c * QB
            out[0, q0:q0 + QB] = o[qs, 0:QB]
            out[1, q0:q0 + QB] = o[qs, QB:2 * QB]
    return out



# revision 3
# speedup vs baseline: 1.0033x; 1.0033x over previous
"""Causal multi-head attention (B=2, T=2048, D=1024, H=16) on 8 TRN2 NeuronCores.

Sharding: core c = (batch b = c//4, head-group g = c%4). Each core owns 4 heads
(= 256 contiguous dims of D) of one batch: Megatron-style tensor parallelism on
heads x data parallelism on batch.

Design (single fully-pipelined phase; everything but span 0's attention is
emitted as "fillers" interleaved into the attention kt loops so the in-order
PE queue has no phase boundaries and never head-of-line blocks):
  - Out-projection reduction via per-q-span 8-way bf16 AllToAll of the
    normalized attention output yT (rank r's territory = 64-col q-block r of
    each 512-q span, for BOTH batches -- SPMD-uniform, no junk shards). Each
    core then out-projects its territory with the full Wo.
  - Score matmuls pack the two heads of an mc-chunk as two concurrent K=64
    row-group tiles into one [128, 2*512] 2-bank PSUM tile; ONE exp per pair.
    The kt loop software-pipelines scores kt+1 ahead of the AV matmuls of kt;
    fillers pop BEFORE the exp-gated AV matmuls so the in-order PE queue keeps
    running independent work while ACT produces exp(kt).
  - AV uses the v_aug 65th-column trick (denominator accumulates as row 64);
    normalization = PE rank-1 broadcast of 1/den + in-place DVE multiply, one
    span behind attention. The 1/den Ln+Exp chains for spans 0/1 are emitted
    into the NEXT span's ACT stream (after its first exp) so they never gate
    the next span's first AV.
  - x is loaded span-column-major (wq + x[:, :512] first) so span-0
    projections start ~5us after the engines arm; a short PE warmup stream
    covers the load window (HAM un-throttle). Remaining projections are
    fillers with even-spread pacing.
  - Out-projection: pair 0 = spans 0+1 stacked (M=128), spread over span 3's
    kt loop. Span 2 po (M=64 per (b,ns)) depends only on A2A(2) and fills the
    span-3 half-A2A flight windows; span 3 po (M=64) splits even/odd kc across
    the two mc half-exchanges so only odd-kc matmuls wait on the final 128KB
    half. bo is folded in on DVE at PSUM evacuation (no seed matmuls). Output
    DMAs ride the gpsimd queue so they never queue behind gather DMAs.

Dtypes: all matmul operands bf16 with fp32 PSUM accumulation; softmax exp(s)
without row-max (scores O(1), scale folded into Wq host-side).
"""

import os
import numpy as np
import ml_dtypes

BF16 = ml_dtypes.bfloat16

B, T, D, H = 2, 2048, 1024, 16
HD = D // H                     # 64
NCORES = 8
GROUPS = 4                      # cores per batch (tensor-parallel degree)
HL = H // GROUPS                # heads per core = 4
DL = D // GROUPS                # dims per core = 256
SP = 512                        # free-dim span per matmul (one PSUM bank, fp32)
QS = T // SP                    # 4 q spans
KT = T // 128                   # 16 k tiles
QB = 64                         # q columns per rank territory per span
SCALE = HD ** -0.5

_CACHE = {}


def _build_program():
    import concourse.bass as bass  # noqa: F401  (registers bass machinery)
    import concourse.tile as tile
    from concourse import bacc, mybir

    f32 = mybir.dt.float32
    bf16 = mybir.dt.bfloat16
    Exp = mybir.ActivationFunctionType.Exp
    Ln = mybir.ActivationFunctionType.Ln

    nc = bacc.Bacc("TRN2", target_bir_lowering=False, debug=False,
                   num_devices=NCORES)

    xT = nc.dram_tensor("xT", [D, T], bf16, kind="ExternalInput")
    wqT = nc.dram_tensor("wqT", [D, DL], bf16, kind="ExternalInput")
    wkT = nc.dram_tensor("wkT", [D, DL], bf16, kind="ExternalInput")
    wvT = nc.dram_tensor("wvT", [D, DL], bf16, kind="ExternalInput")
    woT = nc.dram_tensor("woT", [D, D], bf16, kind="ExternalInput")
    bqP = nc.dram_tensor("bqP", [128, 2], f32, kind="ExternalInput")
    bkP = nc.dram_tensor("bkP", [128, 2], f32, kind="ExternalInput")
    bv = nc.dram_tensor("bv", [1, DL], bf16, kind="ExternalInput")
    bo = nc.dram_tensor("bo", [1, D], f32, kind="ExternalInput")
    maskd = nc.dram_tensor("maskd", [128, 128], bf16, kind="ExternalInput")
    onesb = nc.dram_tensor("onesb", [1, 128], bf16, kind="ExternalInput")
    out_ext = nc.dram_tensor("out", [QS, 128, D], f32, kind="ExternalOutput")

    ALL8 = [[0, 1, 2, 3, 4, 5, 6, 7]]

    with tile.TileContext(nc) as tc:
        with tc.tile_pool(name="main", bufs=1) as main, \
             tc.tile_pool(name="dram", bufs=1, space="DRAM") as dram:
            qT_s = main.tile([128, 2, T], bf16)
            kT_s = main.tile([128, 2, T], bf16)
            v_s = main.tile([128, KT, HL * 65], bf16)
            yT_s = main.tile([128, 2, T], bf16)
            woT_s = main.tile([128, 8, D], bf16)
            bq_s = main.tile([128, 2], f32)
            bk_s = main.tile([128, 2], f32)
            bo_bc = main.tile([128, D], f32)
            onesb_s = main.tile([128, 128], bf16)
            bv_bc = main.tile([128, DL], bf16)
            maskd_s = main.tile([128, 128], bf16)
            warm_s = main.tile([128, 2], f32)
            warm_sb = main.tile([128, SP], bf16)

            # per-span A2A staging (separate tiles avoid false DRAM deps)
            a2a_in = [dram.tile([8, DL, QB], bf16, name=f"a2ain{i}")
                      for i in range(QS)]
            a2a_out = [dram.tile([8, DL, QB], bf16, name=f"a2aout{i}")
                       for i in range(QS)]
            # span 3 exchanges in mc halves (contiguous per-half buffers)
            a2a_in3 = [dram.tile([8, 128, QB], bf16, name=f"a2ain3h{m}")
                       for m in range(2)]
            a2a_out3 = [dram.tile([8, 128, QB], bf16, name=f"a2aout3h{m}")
                        for m in range(2)]

            # PE warmup: back-to-back matmuls on scratch data while the input
            # DMAs stream in, so HAM un-throttles before the first real
            # projection matmul (~6us until wq + x span-0 have landed)
            nc.vector.memset(warm_sb, 1.0)
            with tc.tile_pool(name="warm_psum", bufs=1,
                              space="PSUM") as warm_psum:
                wps = warm_psum.tile([128, SP], f32, tag="w")
                for _ in range(14):
                    nc.tensor.matmul(wps, lhsT=warm_sb[:, 0:128],
                                     rhs=warm_sb, start=True, stop=True)

            # tiny high-priority loads on the sync queue
            nc.sync.dma_start(out=bq_s, in_=bqP[:])
            nc.sync.dma_start(out=bk_s, in_=bkP[:])
            # pre-load the ACT Log+Exp table during phase-1 DMAs so span 0's
            # first real exp doesn't pay the ~2.7us table switch
            nc.scalar.activation(warm_s, warm_sb[:, 0:2], Ln)
            nc.scalar.activation(warm_s, warm_sb[:, 0:2], Exp)
            # small loads on the scalar queue
            nc.scalar.dma_start(out=onesb_s,
                                in_=onesb[:].to_broadcast([128, 128]))
            nc.scalar.dma_start(out=bv_bc, in_=bv[:].to_broadcast([128, DL]))
            nc.scalar.dma_start(out=bo_bc, in_=bo[:].to_broadcast([128, D]))
            # ones column at index 64 of each head's 65-wide block of v_aug
            nc.vector.memset(v_s, 1.0)

            # ---------------- input loads ----------------
            # priority order: wq + x span-0 (span-0 q/k/v proj can start),
            # then wk/wv/maskd, then x spans 1-3, then woT (needed last).
            xt_s = main.tile([128, 8, T], bf16)
            wq_s = main.tile([128, 8, DL], bf16)
            wk_s = main.tile([128, 8, DL], bf16)
            wv_s = main.tile([128, 8, DL], bf16)

            wq_r = wqT[:].rearrange("(c p) n -> c p n", p=128)
            wk_r = wkT[:].rearrange("(c p) n -> c p n", p=128)
            wv_r = wvT[:].rearrange("(c p) n -> c p n", p=128)
            xT_r = xT[:].rearrange("(c p) t -> c p t", p=128)
            woT_r = woT[:].rearrange("(c p) n -> c p n", p=128)
            loads = []
            for c in range(8):
                loads.append((wq_s[:, c, :], wq_r[c]))
            for c in range(8):
                loads.append((xt_s[:, c, 0:SP], xT_r[c][:, 0:SP]))
            for c in range(8):
                loads.append((wk_s[:, c, :], wk_r[c]))
            for c in range(8):
                loads.append((wv_s[:, c, :], wv_r[c]))
            loads.append((maskd_s, maskd[:]))
            for s in (1, 2, 3):
                for c in range(8):
                    loads.append((xt_s[:, c, s * SP:(s + 1) * SP],
                                  xT_r[c][:, s * SP:(s + 1) * SP]))
            for c in range(8):
                loads.append((woT_s[:, c, :], woT_r[c]))
            q3 = [nc.sync, nc.gpsimd, nc.scalar]
            for i, (dst, src) in enumerate(loads):
                q3[i % 3].dma_start(out=dst, in_=src)

            # ---- single pipelined phase ----
            with tc.tile_pool(name="attn_t", bufs=4) as attn_t, \
                 tc.tile_pool(name="nrm", bufs=2) as nrm, \
                 tc.tile_pool(name="op_sb", bufs=4) as op_sb, \
                 tc.tile_pool(name="sc_psum", bufs=2, space="PSUM") as sc_psum, \
                 tc.tile_pool(name="av_psum", bufs=2, space="PSUM") as av_psum, \
                 tc.tile_pool(name="op_psum", bufs=2, space="PSUM") as op_psum:

                def attention_span(qs, fillers, act_fillers,
                                   self_norm=False):
                    # denominator rows at partitions 0/32/64/96 (engine APs
                    # must start 32-aligned); memset keeps unused rows finite
                    den_stack = nrm.tile([97, SP], f32, tag="den")
                    nc.vector.memset(den_stack, 1.0)
                    # 1/den as exp(-log(den)) on ACT, one chain per head pair
                    lg = nrm.tile([97, SP], f32, tag="lg")
                    rec_bf = nrm.tile([97, SP], bf16, tag="recf")
                    nkt = 4 * qs + 4  # causal: later k tiles are all-masked
                    span = slice(qs * SP, (qs + 1) * SP)
                    pace = {"left": 2 * nkt, "carry": 0.0}

                    def pop_fillers():
                        frac = pace["carry"] + len(fillers) / pace["left"]
                        n_pop = int(frac)
                        pace["carry"] = frac - n_pop
                        pace["left"] -= 1
                        for _ in range(min(n_pop, len(fillers))):
                            fillers.pop(0)()

                    for p in range(2):  # head pair = mc chunk p
                        qa = qT_s[0:64, p, span]
                        qb = qT_s[64:128, p, span]
                        ya = av_psum.tile([65, SP], f32, tag="av")
                        yb = av_psum.tile([65, SP], f32, tag="av")

                        def sc_pair(kt):
                            # diagonal tiles: q columns < 128*(kt-4qs) are
                            # fully masked; stream only the valid sub-range
                            j0 = max(0, (kt - 4 * qs) * 128)
                            scp = sc_psum.tile([128, 2 * SP], f32, tag="sc")
                            nc.tensor.matmul(
                                scp[:, j0:SP],
                                lhsT=kT_s[0:64, p, kt * 128:(kt + 1) * 128],
                                rhs=qa[:, j0:SP], start=True, stop=True)
                            nc.tensor.matmul(
                                scp[:, SP + j0:2 * SP],
                                lhsT=kT_s[64:128, p, kt * 128:(kt + 1) * 128],
                                rhs=qb[:, j0:SP], start=True, stop=True)
                            return scp

                        # software pipeline: scores kt+1 are emitted before
                        # the AV matmuls of kt so the in-order PE queue keeps
                        # feeding ACT while AV waits on exp kt
                        scp = sc_pair(0)
                        for kt in range(nkt):
                            atp = attn_t.tile([128, 2 * SP], bf16, tag="at")
                            j0e = max(0, (kt - 4 * qs) * 128)
                            if j0e >= 256:
                                # mostly-masked diagonal tile: two narrow
                                # exps over the valid ranges beat one full-
                                # width one
                                nc.scalar.activation(atp[:, j0e:SP],
                                                     scp[:, j0e:SP], Exp)
                                nc.scalar.activation(
                                    atp[:, SP + j0e:2 * SP],
                                    scp[:, SP + j0e:2 * SP], Exp)
                            else:
                                nc.scalar.activation(atp, scp, Exp)
                            if p == 0 and kt == 0:
                                # previous span's 1/den chain rides here so
                                # it never gates this span's first AV
                                while act_fillers:
                                    act_fillers.pop(0)()
                            if kt >= 4 * qs:
                                # diagonal tile: only its 128-col diagonal
                                # block needs masking and that block is the
                                # same tril(128) for every tile
                                jm = (kt - 4 * qs) * 128
                                nc.vector.tensor_mul(
                                    atp[:, jm:jm + 128],
                                    atp[:, jm:jm + 128], maskd_s)
                                nc.vector.tensor_mul(
                                    atp[:, SP + jm:SP + jm + 128],
                                    atp[:, SP + jm:SP + jm + 128], maskd_s)
                            if kt + 1 < nkt:
                                scp = sc_pair(kt + 1)
                            # independent fillers run while ACT produces
                            # exp(kt); they must precede the exp-gated AVs
                            pop_fillers()
                            j0 = max(0, (kt - 4 * qs) * 128)
                            nc.tensor.matmul(
                                ya[:, j0:SP],
                                lhsT=v_s[:, kt, (2 * p) * 65:
                                             (2 * p + 1) * 65],
                                rhs=atp[:, j0:SP],
                                start=(kt == 0), stop=(kt == nkt - 1))
                            nc.tensor.matmul(
                                yb[:, j0:SP],
                                lhsT=v_s[:, kt, (2 * p + 1) * 65:
                                             (2 * p + 2) * 65],
                                rhs=atp[:, SP + j0:2 * SP],
                                start=(kt == 0), stop=(kt == nkt - 1))
                        # evacuate unnormalized yT' + denominators on DVE so
                        # the PSUM banks free for the next pair
                        nc.vector.tensor_copy(yT_s[0:64, p, span], ya[0:64, :])
                        nc.vector.tensor_copy(yT_s[64:128, p, span],
                                              yb[0:64, :])
                        nc.vector.tensor_copy(
                            den_stack[64 * p:64 * p + 1, :], ya[64:65, :])
                        nc.vector.tensor_copy(
                            den_stack[64 * p + 32:64 * p + 33, :],
                            yb[64:65, :])
                        if self_norm:
                            # last span: per-pair reciprocal chain; pair 0's
                            # rows normalize during pair 1's sweep
                            sl = slice(64 * p, 64 * p + 64 if p == 0 else 97)
                            nc.scalar.activation(lg[sl, :], den_stack[sl, :],
                                                 Ln)
                            nc.scalar.activation(rec_bf[sl, :], lg[sl, :],
                                                 Exp, scale=-1.0)
                            if p == 0:
                                # front-insert: the mc0 normalize + half-A2A
                                # must fire early in pair 1's sweep
                                fillers[0:0] = [norm_h(qs, rec_bf, 0),
                                                norm_h(qs, rec_bf, 1),
                                                stage_a2a_mc(qs, 0)]
                                pace["carry"] += 3.0
                    if self_norm:
                        return rec_bf, None
                    if qs == 2:
                        # span 3 starts with plenty of PE fillers (pair-0
                        # out-projection), so the chain can sit at span end
                        # without gating PE; span-2 norms then pop stall-free
                        nc.scalar.activation(lg, den_stack, Ln)
                        nc.scalar.activation(rec_bf, lg, Exp, scale=-1.0)
                        return rec_bf, None

                    def chain():
                        nc.scalar.activation(lg, den_stack, Ln)
                        nc.scalar.activation(rec_bf, lg, Exp, scale=-1.0)
                    return rec_bf, chain

                def norm_h(qs, rec_bf, h):
                    def f():
                        span = slice(qs * SP, (qs + 1) * SP)
                        mc, r0 = divmod(h, 2)
                        r0 *= 64
                        rb = op_psum.tile([64, SP], f32, tag="op")
                        r0p = 32 * h
                        nc.tensor.matmul(rb,
                                         lhsT=onesb_s[r0p:r0p + 1, 0:64],
                                         rhs=rec_bf[r0p:r0p + 1, :],
                                         start=True, stop=True,
                                         tile_position=(r0p, 0))
                        nc.vector.tensor_mul(yT_s[r0:r0 + 64, mc, span],
                                             yT_s[r0:r0 + 64, mc, span],
                                             rb)
                    return f

                def stage_a2a(qs):
                    def f():
                        span = slice(qs * SP, (qs + 1) * SP)
                        in_r = a2a_in[qs][:].rearrange(
                            "j (two p) q -> two p j q", p=128)
                        for mc in range(2):
                            nc.sync.dma_start(
                                out=in_r[mc],
                                in_=yT_s[:, mc, span].rearrange(
                                    "p (j q) -> p j q", q=QB))
                        nc.gpsimd.collective_compute(
                            "AllToAll", mybir.AluOpType.bypass,
                            replica_groups=ALL8,
                            ins=[a2a_in[qs][:].opt()],
                            outs=[a2a_out[qs][:].opt()])
                    return f

                def stage_a2a_mc(qs, mc):
                    # half exchange: shard rows = the mc dims-half only
                    def f():
                        span = slice(qs * SP, (qs + 1) * SP)
                        nc.sync.dma_start(
                            out=a2a_in3[mc][:].rearrange("j p q -> p j q"),
                            in_=yT_s[:, mc, span].rearrange(
                                "p (j q) -> p j q", q=QB))
                        nc.gpsimd.collective_compute(
                            "AllToAll", mybir.AluOpType.bypass,
                            replica_groups=ALL8,
                            ins=[a2a_in3[mc][:].opt()],
                            outs=[a2a_out3[mc][:].opt()])
                    return f

                def proj_qk(w_s, b_s, dst, mc, s):
                    # qT/kT group: out[dims-chunk mc, t-span s]; bias added
                    # on DVE during the PSUM->SBUF evacuation
                    def f():
                        ps = op_psum.tile([128, SP], f32, tag="op",
                                          name="pj")
                        for kc in range(8):
                            nc.tensor.matmul(
                                ps,
                                lhsT=w_s[:, kc, mc * 128:(mc + 1) * 128],
                                rhs=xt_s[:, kc, s * SP:(s + 1) * SP],
                                start=(kc == 0), stop=(kc == 7))
                        nc.vector.tensor_scalar_add(
                            dst[:, mc, s * SP:(s + 1) * SP],
                            ps, b_s[:, mc:mc + 1])
                    return f

                def proj_v(mt):
                    # v tile in natural [t, d] layout; bias via DVE add into
                    # the 65-stride v_aug slots
                    def f():
                        ps = op_psum.tile([128, SP], f32, tag="op",
                                          name="pjv")
                        for kc in range(8):
                            nc.tensor.matmul(
                                ps[:, 0:DL],
                                lhsT=xt_s[:, kc, mt * 128:(mt + 1) * 128],
                                rhs=wv_s[:, kc, :],
                                start=(kc == 0), stop=(kc == 7))
                        nc.vector.tensor_add(
                            v_s[:, mt, :].rearrange(
                                "p (h d) -> p h d", d=65)[:, :, 0:64],
                            ps[:, 0:DL].rearrange("p (h d) -> p h d", d=64),
                            bv_bc.rearrange("p (h d) -> p h d", d=64))
                    return f

                def proj_span_fillers(s):
                    fs = []
                    for mc in range(2):
                        fs.append(proj_qk(wq_s, bq_s, qT_s, mc, s))
                    for mc in range(2):
                        fs.append(proj_qk(wk_s, bk_s, kT_s, mc, s))
                    for mt in range(4 * s, 4 * s + 4):
                        fs.append(proj_v(mt))
                    return fs

                def outproj_pair0_fillers():
                    """Out-projection for spans (0,1) with M=128 (both spans'
                    64-q territories stacked), split into per-(b,ns) filler
                    chunks; spread over span 3's kt loop."""
                    yg = op_sb.tile([128, 8, 2, 2 * QB], bf16, tag="yg",
                                    bufs=1)

                    def gather(sp):
                        def f():
                            out_r = a2a_out[sp][:].rearrange(
                                "(b j2) (h p) q -> b p (j2 h) q", j2=4, p=128)
                            m = sp % 2
                            for b in range(2):
                                nc.sync.dma_start(
                                    out=yg[:, :, b, m * QB:(m + 1) * QB],
                                    in_=out_r[b])
                        return f

                    def po_mms(b, ns, kcs, box, last=False):
                        def f():
                            if not box:
                                box.append(op_psum.tile(
                                    [128, SP], f32, tag="op", name="po"))
                            po = box[0]
                            for kc in kcs:
                                nc.tensor.matmul(
                                    po, lhsT=yg[:, kc, b, :],
                                    rhs=woT_s[:, kc, ns * SP:(ns + 1) * SP],
                                    start=(kc == 0),
                                    stop=(last and kc == kcs[-1]))
                            if last:
                                ob = op_sb.tile([128, SP], f32, tag="ob")
                                nc.vector.tensor_add(
                                    ob, po, bo_bc[:, ns * SP:(ns + 1) * SP])
                                for m in range(2):
                                    nc.gpsimd.dma_start(
                                        out=out_ext[m, b * 64:(b + 1) * 64,
                                                    ns * SP:(ns + 1) * SP],
                                        in_=ob[m * 64:(m + 1) * 64, :])
                        return f

                    fs = [gather(0), gather(1)]
                    for b in range(2):
                        for ns in range(2):
                            box = []
                            fs.append(po_mms(b, ns, [0, 1, 2], box))
                            fs.append(po_mms(b, ns, [3, 4, 5], box))
                            fs.append(po_mms(b, ns, [6, 7], box, last=True))
                    return fs

                def gather2():
                    # span-2 A2A landed long before the tail; sync-queue slot
                    # before stage_h1 is harmless
                    yg2 = op_sb.tile([128, 8, 2, QB], bf16, tag="yg2",
                                     bufs=1)
                    out_r = a2a_out[2][:].rearrange(
                        "(b j2) (h p) q -> b p (j2 h) q", j2=4, p=128)
                    for b in range(2):
                        nc.sync.dma_start(out=yg2[:, :, b, :], in_=out_r[b])
                    return yg2

                def po_span2_unit(yg2, b, ns):
                    # M=64 out-projection of span 2's territory: depends only
                    # on A2A(2) -> fills the span-3 half-A2A flight windows
                    def f():
                        po = op_psum.tile([64, SP], f32, tag="op", name="po2")
                        for kc in range(8):
                            nc.tensor.matmul(
                                po, lhsT=yg2[:, kc, b, :],
                                rhs=woT_s[:, kc, ns * SP:(ns + 1) * SP],
                                start=(kc == 0), stop=(kc == 7))
                        ob = op_sb.tile([64, SP], f32, tag="ob")
                        nc.vector.tensor_add(
                            ob, po, bo_bc[0:64, ns * SP:(ns + 1) * SP])
                        nc.gpsimd.dma_start(
                            out=out_ext[2, b * 64:(b + 1) * 64,
                                        ns * SP:(ns + 1) * SP],
                            in_=ob)
                    return f

                def gather3_h(yg3, h):
                    # only the kc chunks with kc%2 == h (one half-A2A)
                    def f():
                        out_r = a2a_out3[h][:].rearrange(
                            "(b j2) p q -> b p j2 q", j2=4)
                        for b in range(2):
                            nc.sync.dma_start(
                                out=yg3[:, h::2, b, :], in_=out_r[b])
                    return f

                def po_span3_even(yg3, boxes, b, ns, i):
                    def f():
                        pool = op_psum if i < 2 else sc_psum
                        ptag = "op" if i < 2 else "sc"
                        boxes[(b, ns)] = pool.tile([64, SP], f32, tag=ptag,
                                                   name="po3")
                        po = boxes[(b, ns)]
                        for kc in (0, 2, 4, 6):
                            nc.tensor.matmul(
                                po, lhsT=yg3[:, kc, b, :],
                                rhs=woT_s[:, kc, ns * SP:(ns + 1) * SP],
                                start=(kc == 0), stop=False)
                    return f

                def po_span3_odd(yg3, boxes, b, ns):
                    def f():
                        po = boxes[(b, ns)]
                        for kc in (1, 3, 5, 7):
                            nc.tensor.matmul(
                                po, lhsT=yg3[:, kc, b, :],
                                rhs=woT_s[:, kc, ns * SP:(ns + 1) * SP],
                                start=False, stop=(kc == 7))
                        ob = op_sb.tile([64, SP], f32, tag="ob")
                        nc.vector.tensor_add(
                            ob, po, bo_bc[0:64, ns * SP:(ns + 1) * SP])
                        nc.gpsimd.dma_start(
                            out=out_ext[3, b * 64:(b + 1) * 64,
                                        ns * SP:(ns + 1) * SP],
                            in_=ob)
                    return f

                # prologue: just enough of span 0's projections to start its
                # attention; the rest ride the kt loop as early fillers
                p0 = proj_span_fillers(0)
                for f in [p0[0], p0[2], p0[4], p0[5]]:  # q0, k0, v0, v1
                    f()
                pend = [p0[6], p0[7], p0[1], p0[3]]     # v2, v3, q1, k1
                act_pend = []
                rec = {}
                for qs in range(QS):
                    if qs < 3:
                        # span qs+1's projections drain during span qs; the
                        # previous span's norm/A2A fillers follow the first
                        # two proj chunks (A2A triggers ~1/3 into the span)
                        if qs == 0:
                            pend.extend(proj_span_fillers(1))
                        else:
                            # [q-mc0, q-mc1, norm(qs-1)x4, a2a(qs-1), rest]:
                            # the previous span's A2A triggers ~1/2 into this
                            # span, well before its consumers
                            pf = proj_span_fillers(qs + 1)
                            pend[0:0] = pf[:2]
                            pend.extend(pf[2:])
                    else:
                        # A2A(0)/(1) completed spans ago: pair-0's
                        # out-projection spreads over span 3's kt loop
                        pend.extend(outproj_pair0_fillers())
                    rec[qs], chain = attention_span(
                        qs, pend, act_pend, self_norm=(qs == 3))
                    if qs < 3:
                        nf = [norm_h(qs, rec[qs], h) for h in range(HL)]
                        if qs == 2:
                            # rec(2) is ready at span-3 entry (chain emitted
                            # at span-2 end): norms pop first, A2A(2) early
                            pend[0:0] = nf + [stage_a2a(qs)]
                        else:
                            pend.extend(nf + [stage_a2a(qs)])
                    if chain is not None:
                        act_pend.append(chain)
                for f in pend:
                    f()

                # ---- tail: span-3 normalize mc1, half-A2A, out-projections.
                # po2 u0 runs first so the PE isn't idle while ACT finishes
                # pair-1's reciprocal chain; then norms + stage fire h1.
                yg2 = gather2()
                po2 = [po_span2_unit(yg2, b, ns)
                       for b in range(2) for ns in range(2)]
                po2[0]()
                norm_h(3, rec[3], 2)()
                norm_h(3, rec[3], 3)()
                stage_a2a_mc(3, 1)()
                yg3 = op_sb.tile([128, 8, 2, QB], bf16, tag="yg3", bufs=1)
                gather3_h(yg3, 0)()
                for f in po2[1:]:
                    f()
                boxes = {}
                for i, (b, ns) in enumerate(((0, 0), (0, 1), (1, 0), (1, 1))):
                    po_span3_even(yg3, boxes, b, ns, i)()
                gather3_h(yg3, 1)()
                for b in range(2):
                    for ns in range(2):
                        po_span3_odd(yg3, boxes, b, ns)()

    nc.compile()
    return nc


def _get_program():
    if "nc" not in _CACHE:
        _CACHE["nc"] = _build_program()
    return _CACHE["nc"]


def _make_in_maps(x, mask, Wq, bq, Wk, bk, Wv, bv, Wo, bo):
    x = np.asarray(x, np.float32)
    mask = np.asarray(mask, bool)
    Wq = np.asarray(Wq, np.float32)
    Wk = np.asarray(Wk, np.float32)
    Wv = np.asarray(Wv, np.float32)
    Wo = np.asarray(Wo, np.float32)
    bq = np.asarray(bq, np.float32)
    bk = np.asarray(bk, np.float32)
    bv = np.asarray(bv, np.float32)
    bo = np.asarray(bo, np.float32)

    woT = np.ascontiguousarray(Wo.T).astype(BF16)
    in_maps = []
    per_batch = {}
    for b in range(B):
        xTb = np.ascontiguousarray(x[b].T)
        # the only masking the kernel applies is the 128x128 diagonal
        # block (identical for every diagonal tile of a causal mask)
        md = mask[b, 0].T[0:128, 0:128].astype(np.float32)
        per_batch[b] = (xTb, md)
    for c in range(NCORES):
        b, g = divmod(c, GROUPS)
        sl = slice(g * DL, (g + 1) * DL)
        xTb, md = per_batch[b]
        in_maps.append({
            "xT": xTb.astype(BF16),
            "wqT": np.ascontiguousarray((Wq[sl] * SCALE).T).astype(BF16),
            "wkT": np.ascontiguousarray(Wk[sl].T).astype(BF16),
            "wvT": np.ascontiguousarray(Wv[sl].T).astype(BF16),
            "woT": woT,
            "bqP": np.ascontiguousarray((bq[sl] * SCALE).reshape(2, 128).T),
            "bkP": np.ascontiguousarray(bk[sl].reshape(2, 128).T),
            "bv": bv[sl].reshape(1, DL).astype(BF16),
            "bo": bo.reshape(1, D).astype(np.float32),
            "maskd": md.astype(BF16),
            "onesb": np.ones((1, 128), BF16),
        })
    return in_maps


def _capture_profile(nc, in_maps, tmpdir):
    """Run with NTFF capture and process the profile ourselves. Returns
    (results, exec_time_ns|None)."""
    import glob
    import json
    import re
    import subprocess
    from trn_agent_boot.trn_boot import _ntff_profile_via_ctypes
    from concourse import bass2jax

    hook = _ntff_profile_via_ctypes("/opt/axon/libaxon_pjrt.so")
    if hook is None:
        raise RuntimeError("libaxon_pjrt.so lacks NTFF profile symbols")
    os.makedirs(tmpdir, exist_ok=True)
    with hook(tmpdir, [0]):
        results = bass2jax.run_bass_via_pjrt(nc, in_maps, n_cores=NCORES)

    ntffs = glob.glob(os.path.join(tmpdir, "*_body*-device*.ntff"))
    best = None
    for f in ntffs:
        if re.search(r"executable(\d+)-device000000", f):
            if best is None or os.path.getmtime(f) > os.path.getmtime(best):
                best = f
    if best is None:
        raise RuntimeError(f"no NTFF produced in {tmpdir}")
    neff = re.sub(r"-device\d+-execution-\d+\.ntff$", ".neff", best)
    out_json = os.path.join(tmpdir, "prof.json")
    subprocess.check_call(
        ["neuron-profile", "view", "--ignore-nc-buf-usage", "-s", best,
         "-n", neff, "--output-format=json", f"--output-file={out_json}"],
        cwd=tmpdir)
    summary = json.load(open(out_json))["summary"][0]
    return results, int(summary["total_time"] * 1e9)


def kernel(x, mask, Wq, bq, Wk, bk, Wv, bv, Wo, bo):
    from concourse import bass_utils

    in_maps = _make_in_maps(x, mask, Wq, bq, Wk, bk, Wv, bv, Wo, bo)
    nc = _get_program()

    trace = bool(int(os.environ.get("MHA_TRACE", "0")))
    tmpdir = os.environ.get("MHA_TRACE_DIR") or None
    results = None
    if trace and tmpdir:
        try:
            results, exec_ns = _capture_profile(nc, in_maps, tmpdir)
            _CACHE["last_exec_time_ns"] = exec_ns
        except Exception as e:  # profiling is best-effort
            print(f"profiling unavailable: {type(e).__name__}: {e}")
            results = None
    if results is None:
        results = bass_utils.run_bass_kernel_spmd(
            nc, in_maps, core_ids=list(range(NCORES))).results
        _CACHE.setdefault("last_exec_time_ns", None)

    # core c's out[qs] holds rows (q = qs*512 + c*64 + i) for batch 0
    # (rows 0-63) and batch 1 (rows 64-127)
    out = np.empty((B, T, D), np.float32)
    for c in range(NCORES):
        o = results[c]["out"]
        for qs in range(QS):
            q0 = qs * SP + c * QB
            out[0, q0:q0 + QB] = o[qs, 0:QB]
            out[1, q0:q0 + QB] = o[qs, QB:2 * QB]
    return out


# revision 7
# speedup vs baseline: 1.0143x; 1.0109x over previous
"""Causal multi-head attention (B=2, T=2048, D=1024, H=16) on 8 TRN2 NeuronCores.

Sharding: core c = (batch b = c//4, head-group g = c%4). Each core owns 4 heads
(= 256 contiguous dims of D) of one batch: Megatron-style tensor parallelism on
heads x data parallelism on batch.

Design (single fully-pipelined phase; everything but span 0's attention is
emitted as "fillers" interleaved into the attention kt loops so the in-order
PE queue has no phase boundaries and never head-of-line blocks):
  - Out-projection reduction via per-q-span 8-way bf16 AllToAll of the
    normalized attention output yT (rank r's territory = 64-col q-block r of
    each 512-q span, for BOTH batches -- SPMD-uniform, no junk shards). Each
    core then out-projects its territory with the full Wo.
  - Score matmuls pack the two heads of an mc-chunk as two concurrent K=64
    row-group tiles into one [128, 2*512] 2-bank PSUM tile; ONE exp per pair.
    The kt loop software-pipelines scores kt+1 ahead of the AV matmuls of kt;
    fillers pop BEFORE the exp-gated AV matmuls so the in-order PE queue keeps
    running independent work while ACT produces exp(kt). ACT runs ONLY exp --
    a single table set, loaded once at warmup, zero mid-run table switches.
  - AV uses the v_aug 65th-column trick (denominator accumulates as row 64);
    1/den via the single-instruction DVE approx reciprocal (~18 bits, plenty
    upstream of bf16); normalization = PE rank-1 broadcast of 1/den +
    in-place DVE multiply, one span behind attention.
  - x is loaded first-half-of-T-major, interleaved per-kc-chunk with wq, so
    span-0 projection matmuls start as soon as their kc operands land; a
    short PE warmup stream covers the DMA-arm window (HAM un-throttle). Bulk
    loads ride sync/gpsimd/vector queues -- NEVER the scalar queue, whose
    instruction stream must stay free for the exp activations.
  - Out-projection: pair 0 = spans 0+1 stacked (M=128), spread over span 3's
    kt loop as fillers. Spans 2 and 3 use per-(b,ns) M=64 units: span 2's
    depend only on A2A(2) and fill the span-3 A2A flight; span 3's follow the
    single full-span A2A(3) that fires right after the last normalize. bo is
    folded in on DVE at PSUM evacuation. Output DMAs ride the gpsimd queue so
    they never queue behind gather DMAs.

Dtypes: all matmul operands bf16 with fp32 PSUM accumulation; softmax exp(s)
without row-max (scores O(1), scale folded into Wq host-side).
"""

import os
import numpy as np
import ml_dtypes

BF16 = ml_dtypes.bfloat16

B, T, D, H = 2, 2048, 1024, 16
HD = D // H                     # 64
NCORES = 8
GROUPS = 4                      # cores per batch (tensor-parallel degree)
HL = H // GROUPS                # heads per core = 4
DL = D // GROUPS                # dims per core = 256
SP = 512                        # free-dim span per matmul (one PSUM bank, fp32)
QS = T // SP                    # 4 q spans
KT = T // 128                   # 16 k tiles
QB = 64                         # q columns per rank territory per span
SCALE = HD ** -0.5

_CACHE = {}


def _build_program():
    import concourse.bass as bass  # noqa: F401  (registers bass machinery)
    import concourse.tile as tile
    from concourse import bacc, mybir

    f32 = mybir.dt.float32
    bf16 = mybir.dt.bfloat16
    Exp = mybir.ActivationFunctionType.Exp

    nc = bacc.Bacc("TRN2", target_bir_lowering=False, debug=False,
                   num_devices=NCORES)

    xT = nc.dram_tensor("xT", [D, T], bf16, kind="ExternalInput")
    wqT = nc.dram_tensor("wqT", [D, DL], bf16, kind="ExternalInput")
    wkT = nc.dram_tensor("wkT", [D, DL], bf16, kind="ExternalInput")
    wvT = nc.dram_tensor("wvT", [D, DL], bf16, kind="ExternalInput")
    woT = nc.dram_tensor("woT", [D, D], bf16, kind="ExternalInput")
    bqP = nc.dram_tensor("bqP", [128, 2], f32, kind="ExternalInput")
    bkP = nc.dram_tensor("bkP", [128, 2], f32, kind="ExternalInput")
    bv = nc.dram_tensor("bv", [1, DL], bf16, kind="ExternalInput")
    bo = nc.dram_tensor("bo", [1, D], f32, kind="ExternalInput")
    maskd = nc.dram_tensor("maskd", [128, 128], bf16, kind="ExternalInput")
    onesb = nc.dram_tensor("onesb", [1, 128], bf16, kind="ExternalInput")
    out_ext = nc.dram_tensor("out", [QS, 128, D], f32, kind="ExternalOutput")

    ALL8 = [[0, 1, 2, 3, 4, 5, 6, 7]]

    with tile.TileContext(nc) as tc:
        with tc.tile_pool(name="main", bufs=1) as main, \
             tc.tile_pool(name="dram", bufs=1, space="DRAM") as dram:
            qT_s = main.tile([128, 2, T], bf16)
            kT_s = main.tile([128, 2, T], bf16)
            v_s = main.tile([128, KT, HL * 65], bf16)
            yT_s = main.tile([128, 2, T], bf16)
            woT_s = main.tile([128, 8, D], bf16)
            bq_s = main.tile([128, 2], f32)
            bk_s = main.tile([128, 2], f32)
            bo_bc = main.tile([128, D], f32)
            onesb_s = main.tile([128, 128], bf16)
            bv_bc = main.tile([128, DL], bf16)
            maskd_s = main.tile([128, 128], bf16)
            warm_s = main.tile([128, 2], f32)
            warm_sb = main.tile([128, SP], bf16)

            # per-span A2A staging (separate tiles avoid false DRAM deps)
            a2a_in = [dram.tile([8, DL, QB], bf16, name=f"a2ain{i}")
                      for i in range(QS)]
            a2a_out = [dram.tile([8, DL, QB], bf16, name=f"a2aout{i}")
                       for i in range(QS)]

            # PE warmup: back-to-back matmuls on scratch data while the first
            # input DMAs stream in (HAM un-throttle); real projection matmuls
            # take over as soon as their kc operands land
            nc.vector.memset(warm_sb, 1.0)
            with tc.tile_pool(name="warm_psum", bufs=1,
                              space="PSUM") as warm_psum:
                wps = warm_psum.tile([128, SP], f32, tag="w")
                for _ in range(8):
                    nc.tensor.matmul(wps, lhsT=warm_sb[:, 0:128],
                                     rhs=warm_sb, start=True, stop=True)

            # tiny high-priority loads on the sync queue
            nc.sync.dma_start(out=bq_s, in_=bqP[:])
            nc.sync.dma_start(out=bk_s, in_=bkP[:])
            # pre-load the ACT Exp table (the only set the kernel ever uses)
            nc.scalar.activation(warm_s, warm_sb[:, 0:2], Exp)
            # small loads on the scalar queue
            nc.scalar.dma_start(out=onesb_s,
                                in_=onesb[:].to_broadcast([128, 128]))
            nc.scalar.dma_start(out=bv_bc, in_=bv[:].to_broadcast([128, DL]))
            nc.scalar.dma_start(out=bo_bc, in_=bo[:].to_broadcast([128, D]))
            # ones column at index 64 of each head's 65-wide block of v_aug
            nc.vector.memset(v_s, 1.0)

            # ---------------- input loads ----------------
            xt_s = main.tile([128, 8, T], bf16)
            wq_s = main.tile([128, 8, DL], bf16)
            wk_s = main.tile([128, 8, DL], bf16)
            wv_s = main.tile([128, 8, DL], bf16)

            wq_r = wqT[:].rearrange("(c p) n -> c p n", p=128)
            wk_r = wkT[:].rearrange("(c p) n -> c p n", p=128)
            wv_r = wvT[:].rearrange("(c p) n -> c p n", p=128)
            xT_r = xT[:].rearrange("(c p) t -> c p t", p=128)
            woT_r = woT[:].rearrange("(c p) n -> c p n", p=128)
            # lead: x first half (spans 0-1, 2KB/partition packets)
            # interleaved per-chunk with wq so projection kc operands land
            # progressively; round-robin over sync/gpsimd/scalar
            lead = []
            for c in range(8):
                lead.append((xt_s[:, c, 0:2 * SP], xT_r[c][:, 0:2 * SP]))
                lead.append((wq_s[:, c, :], wq_r[c]))
            q3 = [nc.sync, nc.gpsimd, nc.scalar]
            for i, (dst, src) in enumerate(lead):
                q3[i % 3].dma_start(out=dst, in_=src)
            # mid: wk/wv/maskd on sync+gpsimd (scalar queue must stay free
            # for the exp stream from here on)
            mid = [(wk_s[:, c, :], wk_r[c]) for c in range(8)]
            mid += [(wv_s[:, c, :], wv_r[c]) for c in range(8)]
            mid.append((maskd_s, maskd[:]))
            q2 = [nc.sync, nc.gpsimd]
            for i, (dst, src) in enumerate(mid):
                q2[i % 2].dma_start(out=dst, in_=src)
            # tail loads: x second half + woT on sync/gpsimd (scalar's
            # instruction stream stays free for the exp activations)
            tl = [(xt_s[:, c, 2 * SP:T], xT_r[c][:, 2 * SP:T])
                  for c in range(8)]
            tl += [(woT_s[:, c, :], woT_r[c]) for c in range(8)]
            for i, (dst, src) in enumerate(tl):
                q2[i % 2].dma_start(out=dst, in_=src)

            # ---- single pipelined phase ----
            with tc.tile_pool(name="attn_t", bufs=4) as attn_t, \
                 tc.tile_pool(name="nrm", bufs=2) as nrm, \
                 tc.tile_pool(name="op_sb", bufs=4) as op_sb, \
                 tc.tile_pool(name="sc_psum", bufs=2, space="PSUM") as sc_psum, \
                 tc.tile_pool(name="av_psum", bufs=2, space="PSUM") as av_psum, \
                 tc.tile_pool(name="op_psum", bufs=2, space="PSUM") as op_psum:

                def attention_span(qs, fillers, self_norm=False):
                    # denominator rows at partitions 0/32/64/96 (engine APs
                    # must start 32-aligned); memset keeps unused rows finite
                    den_stack = nrm.tile([97, SP], f32, tag="den")
                    nc.vector.memset(den_stack, 1.0)
                    rec32 = nrm.tile([97, SP], f32, tag="rec32")
                    rec_bf = nrm.tile([97, SP], bf16, tag="recf")
                    nkt = 4 * qs + 4  # causal: later k tiles are all-masked
                    span = slice(qs * SP, (qs + 1) * SP)
                    pace = {"left": 2 * nkt, "carry": 0.0}

                    def pop_fillers():
                        frac = pace["carry"] + len(fillers) / pace["left"]
                        n_pop = int(frac)
                        pace["carry"] = frac - n_pop
                        pace["left"] -= 1
                        for _ in range(min(n_pop, len(fillers))):
                            fillers.pop(0)()

                    for p in range(2):  # head pair = mc chunk p
                        qa = qT_s[0:64, p, span]
                        qb = qT_s[64:128, p, span]
                        ya = av_psum.tile([65, SP], f32, tag="av")
                        yb = av_psum.tile([65, SP], f32, tag="av")

                        def sc_pair(kt):
                            # diagonal tiles: q columns < 128*(kt-4qs) are
                            # fully masked; stream only the valid sub-range
                            j0 = max(0, (kt - 4 * qs) * 128)
                            scp = sc_psum.tile([128, 2 * SP], f32, tag="sc")
                            nc.tensor.matmul(
                                scp[:, j0:SP],
                                lhsT=kT_s[0:64, p, kt * 128:(kt + 1) * 128],
                                rhs=qa[:, j0:SP], start=True, stop=True)
                            nc.tensor.matmul(
                                scp[:, SP + j0:2 * SP],
                                lhsT=kT_s[64:128, p, kt * 128:(kt + 1) * 128],
                                rhs=qb[:, j0:SP], start=True, stop=True)
                            return scp

                        # software pipeline: scores kt+1 are emitted before
                        # the AV matmuls of kt so the in-order PE queue keeps
                        # feeding ACT while AV waits on exp kt
                        scp = sc_pair(0)
                        for kt in range(nkt):
                            atp = attn_t.tile([128, 2 * SP], bf16, tag="at")
                            j0e = max(0, (kt - 4 * qs) * 128)
                            if j0e >= 256:
                                # mostly-masked diagonal tile: two narrow
                                # exps over the valid ranges beat one full-
                                # width one
                                nc.scalar.activation(atp[:, j0e:SP],
                                                     scp[:, j0e:SP], Exp)
                                nc.scalar.activation(
                                    atp[:, SP + j0e:2 * SP],
                                    scp[:, SP + j0e:2 * SP], Exp)
                            else:
                                nc.scalar.activation(atp, scp, Exp)
                            if kt >= 4 * qs:
                                # diagonal tile: only its 128-col diagonal
                                # block needs masking and that block is the
                                # same tril(128) for every tile
                                jm = (kt - 4 * qs) * 128
                                nc.vector.tensor_mul(
                                    atp[:, jm:jm + 128],
                                    atp[:, jm:jm + 128], maskd_s)
                                nc.vector.tensor_mul(
                                    atp[:, SP + jm:SP + jm + 128],
                                    atp[:, SP + jm:SP + jm + 128], maskd_s)
                            if kt + 1 < nkt:
                                scp = sc_pair(kt + 1)
                            # independent fillers run while ACT produces
                            # exp(kt); they must precede the exp-gated AVs
                            pop_fillers()
                            j0 = max(0, (kt - 4 * qs) * 128)
                            nc.tensor.matmul(
                                ya[:, j0:SP],
                                lhsT=v_s[:, kt, (2 * p) * 65:
                                             (2 * p + 1) * 65],
                                rhs=atp[:, j0:SP],
                                start=(kt == 0), stop=(kt == nkt - 1))
                            nc.tensor.matmul(
                                yb[:, j0:SP],
                                lhsT=v_s[:, kt, (2 * p + 1) * 65:
                                             (2 * p + 2) * 65],
                                rhs=atp[:, SP + j0:2 * SP],
                                start=(kt == 0), stop=(kt == nkt - 1))
                        # evacuate unnormalized yT' + denominators on DVE so
                        # the PSUM banks free for the next pair
                        nc.vector.tensor_copy(yT_s[0:64, p, span], ya[0:64, :])
                        nc.vector.tensor_copy(yT_s[64:128, p, span],
                                              yb[0:64, :])
                        nc.vector.tensor_copy(
                            den_stack[64 * p:64 * p + 1, :], ya[64:65, :])
                        nc.vector.tensor_copy(
                            den_stack[64 * p + 32:64 * p + 33, :],
                            yb[64:65, :])
                        if self_norm:
                            # last span: full-width DVE reciprocal per pair
                            # (the custom DVE op wants partition offset 0;
                            # pair-1 rows are memset 1.0 until valid and the
                            # second pass overwrites everything)
                            nc.vector.reciprocal_approx_fast(
                                out=rec32, in_=den_stack)
                            nc.vector.tensor_copy(rec_bf, rec32)
                            if p == 0:
                                # front-insert: the mc0 normalize must fire
                                # early in pair 1's sweep
                                fillers[0:0] = [norm_h(qs, rec_bf, 0),
                                                norm_h(qs, rec_bf, 1)]
                                pace["carry"] += 2.0
                    if not self_norm:
                        # single-instruction DVE reciprocal: no ACT table
                        # switches, no span-boundary exp contention
                        nc.vector.reciprocal_approx_fast(out=rec32,
                                                         in_=den_stack)
                        nc.vector.tensor_copy(rec_bf, rec32)
                    return rec_bf

                def norm_h(qs, rec_bf, h):
                    def f():
                        span = slice(qs * SP, (qs + 1) * SP)
                        mc, r0 = divmod(h, 2)
                        r0 *= 64
                        rb = op_psum.tile([64, SP], f32, tag="op")
                        r0p = 32 * h
                        nc.tensor.matmul(rb,
                                         lhsT=onesb_s[r0p:r0p + 1, 0:64],
                                         rhs=rec_bf[r0p:r0p + 1, :],
                                         start=True, stop=True,
                                         tile_position=(r0p, 0))
                        nc.vector.tensor_mul(yT_s[r0:r0 + 64, mc, span],
                                             yT_s[r0:r0 + 64, mc, span],
                                             rb)
                    return f

                def stage_a2a(qs):
                    def f():
                        span = slice(qs * SP, (qs + 1) * SP)
                        in_r = a2a_in[qs][:].rearrange(
                            "j (two p) q -> two p j q", p=128)
                        for mc in range(2):
                            nc.sync.dma_start(
                                out=in_r[mc],
                                in_=yT_s[:, mc, span].rearrange(
                                    "p (j q) -> p j q", q=QB))
                        nc.gpsimd.collective_compute(
                            "AllToAll", mybir.AluOpType.bypass,
                            replica_groups=ALL8,
                            ins=[a2a_in[qs][:].opt()],
                            outs=[a2a_out[qs][:].opt()])
                    return f

                def proj_qk(w_s, b_s, dst, mc, s):
                    # qT/kT group: out[dims-chunk mc, t-span s]; bias added
                    # on DVE during the PSUM->SBUF evacuation
                    def f():
                        ps = op_psum.tile([128, SP], f32, tag="op",
                                          name="pj")
                        for kc in range(8):
                            nc.tensor.matmul(
                                ps,
                                lhsT=w_s[:, kc, mc * 128:(mc + 1) * 128],
                                rhs=xt_s[:, kc, s * SP:(s + 1) * SP],
                                start=(kc == 0), stop=(kc == 7))
                        nc.vector.tensor_scalar_add(
                            dst[:, mc, s * SP:(s + 1) * SP],
                            ps, b_s[:, mc:mc + 1])
                    return f

                def proj_v(mt):
                    # v tile in natural [t, d] layout; bias via DVE add into
                    # the 65-stride v_aug slots
                    def f():
                        ps = op_psum.tile([128, SP], f32, tag="op",
                                          name="pjv")
                        for kc in range(8):
                            nc.tensor.matmul(
                                ps[:, 0:DL],
                                lhsT=xt_s[:, kc, mt * 128:(mt + 1) * 128],
                                rhs=wv_s[:, kc, :],
                                start=(kc == 0), stop=(kc == 7))
                        nc.vector.tensor_add(
                            v_s[:, mt, :].rearrange(
                                "p (h d) -> p h d", d=65)[:, :, 0:64],
                            ps[:, 0:DL].rearrange("p (h d) -> p h d", d=64),
                            bv_bc.rearrange("p (h d) -> p h d", d=64))
                    return f

                def proj_span_fillers(s):
                    fs = []
                    for mc in range(2):
                        fs.append(proj_qk(wq_s, bq_s, qT_s, mc, s))
                    for mc in range(2):
                        fs.append(proj_qk(wk_s, bk_s, kT_s, mc, s))
                    for mt in range(4 * s, 4 * s + 4):
                        fs.append(proj_v(mt))
                    return fs

                def outproj_pair0_fillers():
                    """Out-projection for spans (0,1) with M=128 (both spans'
                    64-q territories stacked), split into per-(b,ns) filler
                    chunks; spread over span 3's kt loop."""
                    yg = op_sb.tile([128, 8, 2, 2 * QB], bf16, tag="yg",
                                    bufs=1)

                    def gather(sp):
                        def f():
                            out_r = a2a_out[sp][:].rearrange(
                                "(b j2) (h p) q -> b p (j2 h) q", j2=4, p=128)
                            m = sp % 2
                            for b in range(2):
                                nc.sync.dma_start(
                                    out=yg[:, :, b, m * QB:(m + 1) * QB],
                                    in_=out_r[b])
                        return f

                    def po_mms(b, ns, kcs, box, last=False):
                        def f():
                            if not box:
                                box.append(op_psum.tile(
                                    [128, SP], f32, tag="op", name="po"))
                            po = box[0]
                            for kc in kcs:
                                nc.tensor.matmul(
                                    po, lhsT=yg[:, kc, b, :],
                                    rhs=woT_s[:, kc, ns * SP:(ns + 1) * SP],
                                    start=(kc == 0),
                                    stop=(last and kc == kcs[-1]))
                            if last:
                                ob = op_sb.tile([128, SP], f32, tag="ob")
                                nc.vector.tensor_add(
                                    ob, po, bo_bc[:, ns * SP:(ns + 1) * SP])
                                for m in range(2):
                                    nc.gpsimd.dma_start(
                                        out=out_ext[m, b * 64:(b + 1) * 64,
                                                    ns * SP:(ns + 1) * SP],
                                        in_=ob[m * 64:(m + 1) * 64, :])
                        return f

                    fs = [gather(0), gather(1)]
                    for b in range(2):
                        for ns in range(2):
                            box = []
                            fs.append(po_mms(b, ns, [0, 1, 2], box))
                            fs.append(po_mms(b, ns, [3, 4, 5], box))
                            fs.append(po_mms(b, ns, [6, 7], box, last=True))
                    return fs

                def gather_span(sp, tag):
                    yg = op_sb.tile([128, 8, 2, QB], bf16, tag=tag, bufs=1,
                                    name=tag)
                    out_r = a2a_out[sp][:].rearrange(
                        "(b j2) (h p) q -> b p (j2 h) q", j2=4, p=128)
                    for b in range(2):
                        nc.sync.dma_start(out=yg[:, :, b, :], in_=out_r[b])
                    return yg

                def po_unit(sp, yg, b, ns):
                    # M=64 out-projection of span sp's territory
                    def f():
                        po = op_psum.tile([64, SP], f32, tag="op", name="pou")
                        for kc in range(8):
                            nc.tensor.matmul(
                                po, lhsT=yg[:, kc, b, :],
                                rhs=woT_s[:, kc, ns * SP:(ns + 1) * SP],
                                start=(kc == 0), stop=(kc == 7))
                        ob = op_sb.tile([64, SP], f32, tag="ob")
                        nc.vector.tensor_add(
                            ob, po, bo_bc[0:64, ns * SP:(ns + 1) * SP])
                        nc.gpsimd.dma_start(
                            out=out_ext[sp, b * 64:(b + 1) * 64,
                                        ns * SP:(ns + 1) * SP],
                            in_=ob)
                    return f

                # prologue: just enough of span 0's projections to start its
                # attention; the rest ride the kt loop as early fillers
                p0 = proj_span_fillers(0)
                for f in [p0[0], p0[2], p0[4], p0[5]]:  # q0, k0, v0, v1
                    f()
                pend = [p0[6], p0[7], p0[1], p0[3]]     # v2, v3, q1, k1
                rec = {}
                for qs in range(QS):
                    if qs < 3:
                        # span qs+1's projections drain during span qs
                        pend.extend(proj_span_fillers(qs + 1))
                    else:
                        # A2A(0)/(1) completed spans ago: pair-0's
                        # out-projection spreads over span 3's kt loop
                        pend.extend(outproj_pair0_fillers())
                    rec[qs] = attention_span(qs, pend, self_norm=(qs == 3))
                    if qs < 3:
                        nf = [norm_h(qs, rec[qs], h) for h in range(HL)]
                        if qs == 2:
                            # norms/A2A(2) pop first in span 3 so the A2A(2)
                            # flight fully precedes its po consumers
                            pend[0:0] = nf + [stage_a2a(qs)]
                        else:
                            pend.extend(nf + [stage_a2a(qs)])
                for f in pend:
                    f()

                # ---- tail: normalize span-3 mc1, fire the single full-span
                # A2A(3), out-project span 2 during its flight, then span 3.
                norm_h(3, rec[3], 2)()
                norm_h(3, rec[3], 3)()
                stage_a2a(3)()
                yg2 = gather_span(2, "yg2")
                for b in range(2):
                    for ns in range(2):
                        po_unit(2, yg2, b, ns)()
                yg3 = gather_span(3, "yg3")
                for b in range(2):
                    for ns in range(2):
                        po_unit(3, yg3, b, ns)()

    nc.compile()
    return nc


def _get_program():
    if "nc" not in _CACHE:
        _CACHE["nc"] = _build_program()
    return _CACHE["nc"]


def _make_in_maps(x, mask, Wq, bq, Wk, bk, Wv, bv, Wo, bo):
    x = np.asarray(x, np.float32)
    mask = np.asarray(mask, bool)
    Wq = np.asarray(Wq, np.float32)
    Wk = np.asarray(Wk, np.float32)
    Wv = np.asarray(Wv, np.float32)
    Wo = np.asarray(Wo, np.float32)
    bq = np.asarray(bq, np.float32)
    bk = np.asarray(bk, np.float32)
    bv = np.asarray(bv, np.float32)
    bo = np.asarray(bo, np.float32)

    woT = np.ascontiguousarray(Wo.T).astype(BF16)
    in_maps = []
    per_batch = {}
    for b in range(B):
        xTb = np.ascontiguousarray(x[b].T)
        # the only masking the kernel applies is the 128x128 diagonal
        # block (identical for every diagonal tile of a causal mask)
        md = mask[b, 0].T[0:128, 0:128].astype(np.float32)
        per_batch[b] = (xTb, md)
    for c in range(NCORES):
        b, g = divmod(c, GROUPS)
        sl = slice(g * DL, (g + 1) * DL)
        xTb, md = per_batch[b]
        in_maps.append({
            "xT": xTb.astype(BF16),
            "wqT": np.ascontiguousarray((Wq[sl] * SCALE).T).astype(BF16),
            "wkT": np.ascontiguousarray(Wk[sl].T).astype(BF16),
            "wvT": np.ascontiguousarray(Wv[sl].T).astype(BF16),
            "woT": woT,
            "bqP": np.ascontiguousarray((bq[sl] * SCALE).reshape(2, 128).T),
            "bkP": np.ascontiguousarray(bk[sl].reshape(2, 128).T),
            "bv": bv[sl].reshape(1, DL).astype(BF16),
            "bo": bo.reshape(1, D).astype(np.float32),
            "maskd": md.astype(BF16),
            "onesb": np.ones((1, 128), BF16),
        })
    return in_maps


def _capture_profile(nc, in_maps, tmpdir):
    """Run with NTFF capture and process the profile ourselves. Returns
    (results, exec_time_ns|None)."""
    import glob
    import json
    import re
    import subprocess
    from trn_agent_boot.trn_boot import _ntff_profile_via_ctypes
    from concourse import bass2jax

    hook = _ntff_profile_via_ctypes("/opt/axon/libaxon_pjrt.so")
    if hook is None:
        raise RuntimeError("libaxon_pjrt.so lacks NTFF profile symbols")
    os.makedirs(tmpdir, exist_ok=True)
    with hook(tmpdir, [0]):
        results = bass2jax.run_bass_via_pjrt(nc, in_maps, n_cores=NCORES)

    ntffs = glob.glob(os.path.join(tmpdir, "*_body*-device*.ntff"))
    best = None
    for f in ntffs:
        if re.search(r"executable(\d+)-device000000", f):
            if best is None or os.path.getmtime(f) > os.path.getmtime(best):
                best = f
    if best is None:
        raise RuntimeError(f"no NTFF produced in {tmpdir}")
    neff = re.sub(r"-device\d+-execution-\d+\.ntff$", ".neff", best)
    out_json = os.path.join(tmpdir, "prof.json")
    subprocess.check_call(
        ["neuron-profile", "view", "--ignore-nc-buf-usage", "-s", best,
         "-n", neff, "--output-format=json", f"--output-file={out_json}"],
        cwd=tmpdir)
    summary = json.load(open(out_json))["summary"][0]
    return results, int(summary["total_time"] * 1e9)


def kernel(x, mask, Wq, bq, Wk, bk, Wv, bv, Wo, bo):
    from concourse import bass_utils

    in_maps = _make_in_maps(x, mask, Wq, bq, Wk, bk, Wv, bv, Wo, bo)
    nc = _get_program()

    trace = bool(int(os.environ.get("MHA_TRACE", "0")))
    tmpdir = os.environ.get("MHA_TRACE_DIR") or None
    results = None
    if trace and tmpdir:
        try:
            results, exec_ns = _capture_profile(nc, in_maps, tmpdir)
            _CACHE["last_exec_time_ns"] = exec_ns
        except Exception as e:  # profiling is best-effort
            print(f"profiling unavailable: {type(e).__name__}: {e}")
            results = None
    if results is None:
        results = bass_utils.run_bass_kernel_spmd(
            nc, in_maps, core_ids=list(range(NCORES))).results
        _CACHE.setdefault("last_exec_time_ns", None)

    # core c's out[qs] holds rows (q = qs*512 + c*64 + i) for batch 0
    # (rows 0-63) and batch 1 (rows 64-127)
    out = np.empty((B, T, D), np.float32)
    for c in range(NCORES):
        o = results[c]["out"]
        for qs in range(QS):
            q0 = qs * SP + c * QB
            out[0, q0:q0 + QB] = o[qs, 0:QB]
            out[1, q0:q0 + QB] = o[qs, QB:2 * QB]
    return out


# revision 12
# speedup vs baseline: 1.0320x; 1.0175x over previous
"""Causal multi-head attention (B=2, T=2048, D=1024, H=16) on 8 TRN2 NeuronCores.

Sharding: core c = (batch b = c//4, head-group g = c%4). Each core owns 4 heads
(= 256 contiguous dims of D) of one batch: Megatron-style tensor parallelism on
heads x data parallelism on batch.

Design (single fully-pipelined phase; everything but span 0's attention is
emitted as "fillers" interleaved into the attention kt loops so the in-order
PE queue has no phase boundaries and never head-of-line blocks):
  - Out-projection reduction via per-q-span 8-way bf16 AllToAll of the
    normalized attention output yT (rank r's territory = 64-col q-block r of
    each 512-q span, for BOTH batches -- SPMD-uniform, no junk shards). Each
    core then out-projects its territory with the full Wo.
  - Score matmuls pack the two heads of an mc-chunk as two concurrent K=64
    row-group tiles into one [128, 2*512] 2-bank PSUM tile; ONE exp per pair.
    The kt loop software-pipelines scores kt+1 ahead of the AV matmuls of kt;
    fillers pop BEFORE the exp-gated AV matmuls so the in-order PE queue keeps
    running independent work while ACT produces exp(kt). ACT runs ONLY exp --
    a single table set, loaded once at warmup, zero mid-run table switches.
  - AV uses the v_aug 65th-column trick (denominator accumulates as row 64);
    1/den via the single-instruction DVE approx reciprocal (~18 bits, plenty
    upstream of bf16); normalization = PE rank-1 broadcast of 1/den +
    in-place DVE multiply, one span behind attention.
  - x is loaded first-half-of-T-major, interleaved per-kc-chunk with wq, so
    span-0 projection matmuls start as soon as their kc operands land; a
    short PE warmup stream covers the DMA-arm window (HAM un-throttle). Bulk
    loads ride sync/gpsimd/vector queues -- NEVER the scalar queue, whose
    instruction stream must stay free for the exp activations.
  - Out-projection: pair 0 = spans 0+1 stacked (M=128), spread over span 3's
    kt loop as fillers. Spans 2 and 3 use per-(b,ns) M=64 units: span 2's
    depend only on A2A(2) and fill the span-3 A2A flight; span 3's follow the
    single full-span A2A(3) that fires right after the last normalize. bo is
    folded in on DVE at PSUM evacuation. Output DMAs ride the gpsimd queue so
    they never queue behind gather DMAs.

Dtypes: all matmul operands bf16 with fp32 PSUM accumulation; softmax exp(s)
without row-max (scores O(1), scale folded into Wq host-side).
"""

import os
import numpy as np
import ml_dtypes

BF16 = ml_dtypes.bfloat16

B, T, D, H = 2, 2048, 1024, 16
HD = D // H                     # 64
NCORES = 8
GROUPS = 4                      # cores per batch (tensor-parallel degree)
HL = H // GROUPS                # heads per core = 4
DL = D // GROUPS                # dims per core = 256
SP = 512                        # free-dim span per matmul (one PSUM bank, fp32)
QS = T // SP                    # 4 q spans
KT = T // 128                   # 16 k tiles
QB = 64                         # q columns per rank territory per span
SCALE = HD ** -0.5

_CACHE = {}


def _build_program():
    import concourse.bass as bass  # noqa: F401  (registers bass machinery)
    import concourse.tile as tile
    from concourse import bacc, mybir

    f32 = mybir.dt.float32
    bf16 = mybir.dt.bfloat16
    Exp = mybir.ActivationFunctionType.Exp

    nc = bacc.Bacc("TRN2", target_bir_lowering=False, debug=False,
                   num_devices=NCORES)

    xT = nc.dram_tensor("xT", [D, T], bf16, kind="ExternalInput")
    wqT = nc.dram_tensor("wqT", [D, DL], bf16, kind="ExternalInput")
    wkT = nc.dram_tensor("wkT", [D, DL], bf16, kind="ExternalInput")
    wvT = nc.dram_tensor("wvT", [D, DL], bf16, kind="ExternalInput")
    woT = nc.dram_tensor("woT", [D, D], bf16, kind="ExternalInput")
    bqP = nc.dram_tensor("bqP", [128, 2], f32, kind="ExternalInput")
    bkP = nc.dram_tensor("bkP", [128, 2], f32, kind="ExternalInput")
    bv = nc.dram_tensor("bv", [1, DL], bf16, kind="ExternalInput")
    bo = nc.dram_tensor("bo", [1, D], f32, kind="ExternalInput")
    maskd = nc.dram_tensor("maskd", [128, 128], bf16, kind="ExternalInput")
    onesb = nc.dram_tensor("onesb", [1, 128], bf16, kind="ExternalInput")
    out_ext = nc.dram_tensor("out", [QS, 128, D], f32, kind="ExternalOutput")

    ALL8 = [[0, 1, 2, 3, 4, 5, 6, 7]]

    with tile.TileContext(nc) as tc:
        with tc.tile_pool(name="main", bufs=1) as main, \
             tc.tile_pool(name="dram", bufs=1, space="DRAM") as dram:
            qT_s = main.tile([128, 2, T], bf16)
            kT_s = main.tile([128, 2, T], bf16)
            v_s = main.tile([128, KT, HL * 65], bf16)
            yT_s = main.tile([128, 2, T], bf16)
            woT_s = main.tile([128, 8, D], bf16)
            bq_s = main.tile([128, 2], f32)
            bk_s = main.tile([128, 2], f32)
            bo_bc = main.tile([128, D], f32)
            onesb_s = main.tile([128, 128], bf16)
            bv_bc = main.tile([128, DL], bf16)
            maskd_s = main.tile([128, 128], bf16)
            warm_s = main.tile([128, 2], f32)
            warm_sb = main.tile([128, SP], bf16)

            # per-span A2A staging (separate tiles avoid false DRAM deps)
            a2a_in = [dram.tile([8, DL, QB], bf16, name=f"a2ain{i}")
                      for i in range(QS)]
            a2a_out = [dram.tile([8, DL, QB], bf16, name=f"a2aout{i}")
                       for i in range(QS)]

            # PE warmup: back-to-back matmuls on scratch data while the first
            # input DMAs stream in (HAM un-throttle); real projection matmuls
            # take over as soon as their kc operands land
            nc.vector.memset(warm_sb, 1.0)
            with tc.tile_pool(name="warm_psum", bufs=1,
                              space="PSUM") as warm_psum:
                wps = warm_psum.tile([128, SP], f32, tag="w")
                for _ in range(12):
                    nc.tensor.matmul(wps, lhsT=warm_sb[:, 0:128],
                                     rhs=warm_sb, start=True, stop=True)

            # tiny high-priority loads on the sync queue
            nc.sync.dma_start(out=bq_s, in_=bqP[:])
            nc.sync.dma_start(out=bk_s, in_=bkP[:])
            # pre-load the ACT Exp table (the only set the kernel ever uses)
            nc.scalar.activation(warm_s, warm_sb[:, 0:2], Exp)
            # small loads on the scalar queue
            nc.scalar.dma_start(out=onesb_s,
                                in_=onesb[:].to_broadcast([128, 128]))
            nc.scalar.dma_start(out=bv_bc, in_=bv[:].to_broadcast([128, DL]))
            nc.scalar.dma_start(out=bo_bc, in_=bo[:].to_broadcast([128, D]))
            # ones column at index 64 of each head's 65-wide block of v_aug
            nc.vector.memset(v_s, 1.0)

            # ---------------- input loads ----------------
            xt_s = main.tile([128, 8, T], bf16)
            wq_s = main.tile([128, 8, DL], bf16)
            wk_s = main.tile([128, 8, DL], bf16)
            wv_s = main.tile([128, 8, DL], bf16)

            wq_r = wqT[:].rearrange("(c p) n -> c p n", p=128)
            wk_r = wkT[:].rearrange("(c p) n -> c p n", p=128)
            wv_r = wvT[:].rearrange("(c p) n -> c p n", p=128)
            xT_r = xT[:].rearrange("(c p) t -> c p t", p=128)
            woT_r = woT[:].rearrange("(c p) n -> c p n", p=128)
            # lead: x first half (spans 0-1, 2KB/partition packets)
            # interleaved per-chunk with wq so projection kc operands land
            # progressively; round-robin over sync/gpsimd/scalar
            lead = []
            for c in range(3):
                lead.append((xt_s[:, c, 0:2 * SP], xT_r[c][:, 0:2 * SP]))
            for c in range(3):
                lead.append((wq_s[:, c, :], wq_r[c]))
            for c in range(3, 8):
                lead.append((xt_s[:, c, 0:2 * SP], xT_r[c][:, 0:2 * SP]))
                lead.append((wq_s[:, c, :], wq_r[c]))
            q3 = [nc.sync, nc.gpsimd, nc.scalar]
            for i, (dst, src) in enumerate(lead):
                q3[i % 3].dma_start(out=dst, in_=src)
            # mid: wk/wv/maskd on sync+gpsimd (scalar queue must stay free
            # for the exp stream from here on)
            mid = [(wk_s[:, c, :], wk_r[c]) for c in range(8)]
            mid += [(wv_s[:, c, :], wv_r[c]) for c in range(8)]
            mid.append((maskd_s, maskd[:]))
            q2 = [nc.sync, nc.gpsimd]
            for i, (dst, src) in enumerate(mid):
                q2[i % 2].dma_start(out=dst, in_=src)
            # tail loads: x second half + woT on sync/gpsimd (scalar's
            # instruction stream stays free for the exp activations)
            tl = [(xt_s[:, c, 2 * SP:T], xT_r[c][:, 2 * SP:T])
                  for c in range(8)]
            tl += [(woT_s[:, c, :], woT_r[c]) for c in range(8)]
            for i, (dst, src) in enumerate(tl):
                q2[i % 2].dma_start(out=dst, in_=src)

            # ---- single pipelined phase ----
            with tc.tile_pool(name="attn_t", bufs=4) as attn_t, \
                 tc.tile_pool(name="nrm", bufs=2) as nrm, \
                 tc.tile_pool(name="op_sb", bufs=4) as op_sb, \
                 tc.tile_pool(name="sc_psum", bufs=2, space="PSUM") as sc_psum, \
                 tc.tile_pool(name="av_psum", bufs=2, space="PSUM") as av_psum, \
                 tc.tile_pool(name="op_psum", bufs=2, space="PSUM") as op_psum:

                def attention_span(qs, fillers, self_norm=False):
                    # denominator rows at partitions 0/32/64/96 (engine APs
                    # must start 32-aligned); memset keeps unused rows finite
                    den_stack = nrm.tile([97, SP], f32, tag="den")
                    nc.vector.memset(den_stack, 1.0)
                    rec32 = nrm.tile([97, SP], f32, tag="rec32")
                    rec_bf = nrm.tile([97, SP], bf16, tag="recf")
                    nkt = 4 * qs + 4  # causal: later k tiles are all-masked
                    span = slice(qs * SP, (qs + 1) * SP)
                    pace = {"left": 2 * nkt, "carry": 0.0}

                    def pop_fillers():
                        frac = pace["carry"] + len(fillers) / pace["left"]
                        n_pop = int(frac)
                        pace["carry"] = frac - n_pop
                        pace["left"] -= 1
                        for _ in range(min(n_pop, len(fillers))):
                            fillers.pop(0)()

                    for p in range(2):  # head pair = mc chunk p
                        qa = qT_s[0:64, p, span]
                        qb = qT_s[64:128, p, span]
                        ya = av_psum.tile([65, SP], f32, tag="av")
                        yb = av_psum.tile([65, SP], f32, tag="av")

                        def sc_pair(kt):
                            # diagonal tiles: q columns < 128*(kt-4qs) are
                            # fully masked; stream only the valid sub-range
                            j0 = max(0, (kt - 4 * qs) * 128)
                            scp = sc_psum.tile([128, 2 * SP], f32, tag="sc")
                            nc.tensor.matmul(
                                scp[:, j0:SP],
                                lhsT=kT_s[0:64, p, kt * 128:(kt + 1) * 128],
                                rhs=qa[:, j0:SP], start=True, stop=True)
                            nc.tensor.matmul(
                                scp[:, SP + j0:2 * SP],
                                lhsT=kT_s[64:128, p, kt * 128:(kt + 1) * 128],
                                rhs=qb[:, j0:SP], start=True, stop=True)
                            return scp

                        # software pipeline: scores kt+1 are emitted before
                        # the AV matmuls of kt so the in-order PE queue keeps
                        # feeding ACT while AV waits on exp kt
                        scp = sc_pair(0)
                        for kt in range(nkt):
                            atp = attn_t.tile([128, 2 * SP], bf16, tag="at")
                            j0e = max(0, (kt - 4 * qs) * 128)
                            if j0e >= 256:
                                # mostly-masked diagonal tile: two narrow
                                # exps over the valid ranges beat one full-
                                # width one
                                nc.scalar.activation(atp[:, j0e:SP],
                                                     scp[:, j0e:SP], Exp)
                                nc.scalar.activation(
                                    atp[:, SP + j0e:2 * SP],
                                    scp[:, SP + j0e:2 * SP], Exp)
                            else:
                                nc.scalar.activation(atp, scp, Exp)
                            if kt >= 4 * qs:
                                # diagonal tile: only its 128-col diagonal
                                # block needs masking and that block is the
                                # same tril(128) for every tile
                                jm = (kt - 4 * qs) * 128
                                nc.vector.tensor_mul(
                                    atp[:, jm:jm + 128],
                                    atp[:, jm:jm + 128], maskd_s)
                                nc.vector.tensor_mul(
                                    atp[:, SP + jm:SP + jm + 128],
                                    atp[:, SP + jm:SP + jm + 128], maskd_s)
                            if kt + 1 < nkt:
                                scp = sc_pair(kt + 1)
                            # independent fillers run while ACT produces
                            # exp(kt); they must precede the exp-gated AVs
                            pop_fillers()
                            j0 = max(0, (kt - 4 * qs) * 128)
                            nc.tensor.matmul(
                                ya[:, j0:SP],
                                lhsT=v_s[:, kt, (2 * p) * 65:
                                             (2 * p + 1) * 65],
                                rhs=atp[:, j0:SP],
                                start=(kt == 0), stop=(kt == nkt - 1))
                            nc.tensor.matmul(
                                yb[:, j0:SP],
                                lhsT=v_s[:, kt, (2 * p + 1) * 65:
                                             (2 * p + 2) * 65],
                                rhs=atp[:, SP + j0:2 * SP],
                                start=(kt == 0), stop=(kt == nkt - 1))
                        # evacuate unnormalized yT' + denominators so the
                        # PSUM banks free for the next pair; the last pair's
                        # evacuation is on the A2A(3)-trigger critical path,
                        # so split it across DVE and the idle ACT engine
                        if self_norm and p == 1:
                            nc.vector.tensor_copy(yT_s[0:64, p, span],
                                                  ya[0:64, :])
                            nc.scalar.copy(yT_s[64:128, p, span],
                                           yb[0:64, :])
                            nc.scalar.copy(
                                den_stack[64 * p:64 * p + 1, :], ya[64:65, :])
                            nc.scalar.copy(
                                den_stack[64 * p + 32:64 * p + 33, :],
                                yb[64:65, :])
                        else:
                            nc.vector.tensor_copy(yT_s[0:64, p, span],
                                                  ya[0:64, :])
                            nc.vector.tensor_copy(yT_s[64:128, p, span],
                                                  yb[0:64, :])
                            nc.vector.tensor_copy(
                                den_stack[64 * p:64 * p + 1, :], ya[64:65, :])
                            nc.vector.tensor_copy(
                                den_stack[64 * p + 32:64 * p + 33, :],
                                yb[64:65, :])
                        if self_norm:
                            # last span: full-width DVE reciprocal per pair
                            # (the custom DVE op wants partition offset 0;
                            # pair-1 rows are memset 1.0 until valid and the
                            # second pass overwrites everything)
                            nc.vector.reciprocal_approx_fast(
                                out=rec32, in_=den_stack)
                            nc.vector.tensor_copy(rec_bf, rec32)
                            if p == 0:
                                # front-insert: the mc0 normalize must fire
                                # early in pair 1's sweep
                                fillers[0:0] = [norm_h(qs, rec_bf, 0),
                                                norm_h(qs, rec_bf, 1)]
                                pace["carry"] += 2.0
                    if not self_norm:
                        # single-instruction DVE reciprocal: no ACT table
                        # switches, no span-boundary exp contention
                        nc.vector.reciprocal_approx_fast(out=rec32,
                                                         in_=den_stack)
                        nc.vector.tensor_copy(rec_bf, rec32)
                    return rec_bf

                def norm_h(qs, rec_bf, h):
                    def f():
                        span = slice(qs * SP, (qs + 1) * SP)
                        mc, r0 = divmod(h, 2)
                        r0 *= 64
                        rb = op_psum.tile([64, SP], f32, tag="op")
                        r0p = 32 * h
                        nc.tensor.matmul(rb,
                                         lhsT=onesb_s[r0p:r0p + 1, 0:64],
                                         rhs=rec_bf[r0p:r0p + 1, :],
                                         start=True, stop=True,
                                         tile_position=(r0p, 0))
                        nc.vector.tensor_mul(yT_s[r0:r0 + 64, mc, span],
                                             yT_s[r0:r0 + 64, mc, span],
                                             rb)
                    return f

                def stage_a2a(qs):
                    def f():
                        span = slice(qs * SP, (qs + 1) * SP)
                        in_r = a2a_in[qs][:].rearrange(
                            "j (two p) q -> two p j q", p=128)
                        for mc in range(2):
                            nc.sync.dma_start(
                                out=in_r[mc],
                                in_=yT_s[:, mc, span].rearrange(
                                    "p (j q) -> p j q", q=QB))
                        nc.gpsimd.collective_compute(
                            "AllToAll", mybir.AluOpType.bypass,
                            replica_groups=ALL8,
                            ins=[a2a_in[qs][:].opt()],
                            outs=[a2a_out[qs][:].opt()])
                    return f

                def proj_qk(w_s, b_s, dst, mc, s):
                    # qT/kT group: out[dims-chunk mc, t-span s]; bias added
                    # on DVE during the PSUM->SBUF evacuation
                    def f():
                        ps = op_psum.tile([128, SP], f32, tag="op",
                                          name="pj")
                        for kc in range(8):
                            nc.tensor.matmul(
                                ps,
                                lhsT=w_s[:, kc, mc * 128:(mc + 1) * 128],
                                rhs=xt_s[:, kc, s * SP:(s + 1) * SP],
                                start=(kc == 0), stop=(kc == 7))
                        nc.vector.tensor_scalar_add(
                            dst[:, mc, s * SP:(s + 1) * SP],
                            ps, b_s[:, mc:mc + 1])
                    return f

                def proj_v(mt):
                    # v tile in natural [t, d] layout; bias via DVE add into
                    # the 65-stride v_aug slots
                    def f():
                        ps = op_psum.tile([128, SP], f32, tag="op",
                                          name="pjv")
                        for kc in range(8):
                            nc.tensor.matmul(
                                ps[:, 0:DL],
                                lhsT=xt_s[:, kc, mt * 128:(mt + 1) * 128],
                                rhs=wv_s[:, kc, :],
                                start=(kc == 0), stop=(kc == 7))
                        nc.vector.tensor_add(
                            v_s[:, mt, :].rearrange(
                                "p (h d) -> p h d", d=65)[:, :, 0:64],
                            ps[:, 0:DL].rearrange("p (h d) -> p h d", d=64),
                            bv_bc.rearrange("p (h d) -> p h d", d=64))
                    return f

                def proj_span_fillers(s):
                    fs = []
                    for mc in range(2):
                        fs.append(proj_qk(wq_s, bq_s, qT_s, mc, s))
                    for mc in range(2):
                        fs.append(proj_qk(wk_s, bk_s, kT_s, mc, s))
                    for mt in range(4 * s, 4 * s + 4):
                        fs.append(proj_v(mt))
                    return fs

                def outproj_pair0_fillers():
                    """Out-projection for spans (0,1) with M=128 (both spans'
                    64-q territories stacked), split into per-(b,ns) filler
                    chunks; spread over span 3's kt loop."""
                    yg = op_sb.tile([128, 8, 2, 2 * QB], bf16, tag="yg",
                                    bufs=1)

                    def gather(sp):
                        def f():
                            out_r = a2a_out[sp][:].rearrange(
                                "(b j2) (h p) q -> b p (j2 h) q", j2=4, p=128)
                            m = sp % 2
                            for b in range(2):
                                nc.sync.dma_start(
                                    out=yg[:, :, b, m * QB:(m + 1) * QB],
                                    in_=out_r[b])
                        return f

                    def po_mms(b, ns, kcs, box, last=False):
                        def f():
                            if not box:
                                box.append(op_psum.tile(
                                    [128, SP], f32, tag="op", name="po"))
                            po = box[0]
                            for kc in kcs:
                                nc.tensor.matmul(
                                    po, lhsT=yg[:, kc, b, :],
                                    rhs=woT_s[:, kc, ns * SP:(ns + 1) * SP],
                                    start=(kc == 0),
                                    stop=(last and kc == kcs[-1]))
                            if last:
                                ob = op_sb.tile([128, SP], f32, tag="ob")
                                nc.vector.tensor_add(
                                    ob, po, bo_bc[:, ns * SP:(ns + 1) * SP])
                                for m in range(2):
                                    nc.gpsimd.dma_start(
                                        out=out_ext[m, b * 64:(b + 1) * 64,
                                                    ns * SP:(ns + 1) * SP],
                                        in_=ob[m * 64:(m + 1) * 64, :])
                        return f

                    fs = [gather(0), gather(1)]
                    for b in range(2):
                        for ns in range(2):
                            box = []
                            fs.append(po_mms(b, ns, [0, 1, 2], box))
                            fs.append(po_mms(b, ns, [3, 4, 5], box))
                            fs.append(po_mms(b, ns, [6, 7], box, last=True))
                    return fs

                def gather_span(sp, tag):
                    yg = op_sb.tile([128, 8, 2, QB], bf16, tag=tag, bufs=1,
                                    name=tag)
                    out_r = a2a_out[sp][:].rearrange(
                        "(b j2) (h p) q -> b p (j2 h) q", j2=4, p=128)
                    for b in range(2):
                        nc.sync.dma_start(out=yg[:, :, b, :], in_=out_r[b])
                    return yg

                def po_unit(sp, yg, b, ns):
                    # M=64 out-projection of span sp's territory
                    def f():
                        po = op_psum.tile([64, SP], f32, tag="op", name="pou")
                        for kc in range(8):
                            nc.tensor.matmul(
                                po, lhsT=yg[:, kc, b, :],
                                rhs=woT_s[:, kc, ns * SP:(ns + 1) * SP],
                                start=(kc == 0), stop=(kc == 7))
                        ob = op_sb.tile([64, SP], f32, tag="ob")
                        nc.vector.tensor_add(
                            ob, po, bo_bc[0:64, ns * SP:(ns + 1) * SP])
                        nc.gpsimd.dma_start(
                            out=out_ext[sp, b * 64:(b + 1) * 64,
                                        ns * SP:(ns + 1) * SP],
                            in_=ob)
                    return f

                # prologue: just enough of span 0's projections to start its
                # attention; the rest ride the kt loop as early fillers
                p0 = proj_span_fillers(0)
                for f in [p0[0], p0[2], p0[4], p0[5]]:  # q0, k0, v0, v1
                    f()
                pend = [p0[6], p0[7], p0[1], p0[3]]     # v2, v3, q1, k1
                rec = {}
                for qs in range(QS):
                    if qs < 3:
                        # span qs+1's projections drain during span qs
                        pend.extend(proj_span_fillers(qs + 1))
                    else:
                        # A2A(0)/(1) completed spans ago: pair-0's b=0
                        # out-projection spreads over span 3's kt loop; the
                        # b=1 units are deferred into the A2A(3) flight
                        opf = outproj_pair0_fillers()
                        pend.extend(opf[:8])
                    rec[qs] = attention_span(qs, pend, self_norm=(qs == 3))
                    if qs < 3:
                        nf = [norm_h(qs, rec[qs], h) for h in range(HL)]
                        if qs == 2:
                            # norms/A2A(2) pop first in span 3 so the A2A(2)
                            # flight fully precedes its po consumers
                            pend[0:0] = nf + [stage_a2a(qs)]
                        else:
                            pend.extend(nf + [stage_a2a(qs)])
                for f in pend:
                    f()

                # ---- tail: normalize span-3 mc1, fire the single full-span
                # A2A(3), then fill its ~20us flight with span-2's
                # out-projection, pair-0's deferred b=1 units, and a short
                # dummy-matmul bridge (keeps HAM at full clock so the
                # A2A-gated span-3 out-projection runs warm).
                norm_h(3, rec[3], 2)()
                norm_h(3, rec[3], 3)()
                stage_a2a(3)()
                yg2 = gather_span(2, "yg2")
                for b in range(2):
                    for ns in range(2):
                        po_unit(2, yg2, b, ns)()
                for f in opf[8:]:
                    f()
                dps = sc_psum.tile([128, SP], f32, tag="sc", name="dummy")
                for _ in range(16):
                    nc.tensor.matmul(dps, lhsT=warm_sb[:, 0:128],
                                     rhs=warm_sb, start=True, stop=True)
                yg3 = gather_span(3, "yg3")
                for b in range(2):
                    for ns in range(2):
                        po_unit(3, yg3, b, ns)()

    nc.compile()
    return nc


def _get_program():
    if "nc" not in _CACHE:
        _CACHE["nc"] = _build_program()
    return _CACHE["nc"]


def _make_in_maps(x, mask, Wq, bq, Wk, bk, Wv, bv, Wo, bo):
    x = np.asarray(x, np.float32)
    mask = np.asarray(mask, bool)
    Wq = np.asarray(Wq, np.float32)
    Wk = np.asarray(Wk, np.float32)
    Wv = np.asarray(Wv, np.float32)
    Wo = np.asarray(Wo, np.float32)
    bq = np.asarray(bq, np.float32)
    bk = np.asarray(bk, np.float32)
    bv = np.asarray(bv, np.float32)
    bo = np.asarray(bo, np.float32)

    woT = np.ascontiguousarray(Wo.T).astype(BF16)
    in_maps = []
    per_batch = {}
    for b in range(B):
        xTb = np.ascontiguousarray(x[b].T)
        # the only masking the kernel applies is the 128x128 diagonal
        # block (identical for every diagonal tile of a causal mask)
        md = mask[b, 0].T[0:128, 0:128].astype(np.float32)
        per_batch[b] = (xTb, md)
    for c in range(NCORES):
        b, g = divmod(c, GROUPS)
        sl = slice(g * DL, (g + 1) * DL)
        xTb, md = per_batch[b]
        in_maps.append({
            "xT": xTb.astype(BF16),
            "wqT": np.ascontiguousarray((Wq[sl] * SCALE).T).astype(BF16),
            "wkT": np.ascontiguousarray(Wk[sl].T).astype(BF16),
            "wvT": np.ascontiguousarray(Wv[sl].T).astype(BF16),
            "woT": woT,
            "bqP": np.ascontiguousarray((bq[sl] * SCALE).reshape(2, 128).T),
            "bkP": np.ascontiguousarray(bk[sl].reshape(2, 128).T),
            "bv": bv[sl].reshape(1, DL).astype(BF16),
            "bo": bo.reshape(1, D).astype(np.float32),
            "maskd": md.astype(BF16),
            "onesb": np.ones((1, 128), BF16),
        })
    return in_maps


def _capture_profile(nc, in_maps, tmpdir):
    """Run with NTFF capture and process the profile ourselves. Returns
    (results, exec_time_ns|None)."""
    import glob
    import json
    import re
    import subprocess
    from trn_agent_boot.trn_boot import _ntff_profile_via_ctypes
    from concourse import bass2jax

    hook = _ntff_profile_via_ctypes("/opt/axon/libaxon_pjrt.so")
    if hook is None:
        raise RuntimeError("libaxon_pjrt.so lacks NTFF profile symbols")
    os.makedirs(tmpdir, exist_ok=True)
    with hook(tmpdir, [0]):
        results = bass2jax.run_bass_via_pjrt(nc, in_maps, n_cores=NCORES)

    ntffs = glob.glob(os.path.join(tmpdir, "*_body*-device*.ntff"))
    best = None
    for f in ntffs:
        if re.search(r"executable(\d+)-device000000", f):
            if best is None or os.path.getmtime(f) > os.path.getmtime(best):
                best = f
    if best is None:
        raise RuntimeError(f"no NTFF produced in {tmpdir}")
    neff = re.sub(r"-device\d+-execution-\d+\.ntff$", ".neff", best)
    out_json = os.path.join(tmpdir, "prof.json")
    subprocess.check_call(
        ["neuron-profile", "view", "--ignore-nc-buf-usage", "-s", best,
         "-n", neff, "--output-format=json", f"--output-file={out_json}"],
        cwd=tmpdir)
    summary = json.load(open(out_json))["summary"][0]
    return results, int(summary["total_time"] * 1e9)


def kernel(x, mask, Wq, bq, Wk, bk, Wv, bv, Wo, bo):
    from concourse import bass_utils

    in_maps = _make_in_maps(x, mask, Wq, bq, Wk, bk, Wv, bv, Wo, bo)
    nc = _get_program()

    trace = bool(int(os.environ.get("MHA_TRACE", "0")))
    tmpdir = os.environ.get("MHA_TRACE_DIR") or None
    results = None
    if trace and tmpdir:
        try:
            results, exec_ns = _capture_profile(nc, in_maps, tmpdir)
            _CACHE["last_exec_time_ns"] = exec_ns
        except Exception as e:  # profiling is best-effort
            print(f"profiling unavailable: {type(e).__name__}: {e}")
            results = None
    if results is None:
        results = bass_utils.run_bass_kernel_spmd(
            nc, in_maps, core_ids=list(range(NCORES))).results
        _CACHE.setdefault("last_exec_time_ns", None)

    # core c's out[qs] holds rows (q = qs*512 + c*64 + i) for batch 0
    # (rows 0-63) and batch 1 (rows 64-127)
    out = np.empty((B, T, D), np.float32)
    for c in range(NCORES):
        o = results[c]["out"]
        for qs in range(QS):
            q0 = qs * SP + c * QB
            out[0, q0:q0 + QB] = o[qs, 0:QB]
            out[1, q0:q0 + QB] = o[qs, QB:2 * QB]
    return out


# revision 15
# speedup vs baseline: 1.0492x; 1.0167x over previous
"""Causal multi-head attention (B=2, T=2048, D=1024, H=16) on 8 TRN2 NeuronCores.

Sharding: core c = (batch b = c//4, head-group g = c%4). Each core owns 4 heads
(= 256 contiguous dims of D) of one batch: Megatron-style tensor parallelism on
heads x data parallelism on batch.

Design (single fully-pipelined phase; everything but span 0's attention is
emitted as "fillers" interleaved into the attention kt loops so the in-order
PE queue has no phase boundaries and never head-of-line blocks):
  - Out-projection reduction via per-q-span 8-way bf16 AllToAll of the
    normalized attention output yT (rank r's territory = 64-col q-block r of
    each 512-q span, for BOTH batches -- SPMD-uniform, no junk shards). Each
    core then out-projects its territory with the full Wo.
  - Score matmuls pack the two heads of an mc-chunk as two concurrent K=64
    row-group tiles into one [128, 2*512] 2-bank PSUM tile; ONE exp per pair.
    The kt loop software-pipelines scores kt+1 ahead of the AV matmuls of kt;
    fillers pop BEFORE the exp-gated AV matmuls so the in-order PE queue keeps
    running independent work while ACT produces exp(kt). ACT runs ONLY exp --
    a single table set, loaded once at warmup, zero mid-run table switches.
  - AV uses the v_aug 65th-column trick (denominator accumulates as row 64);
    1/den via the single-instruction DVE approx reciprocal (~18 bits, plenty
    upstream of bf16); normalization = PE rank-1 broadcast of 1/den +
    in-place DVE multiply, one span behind attention.
  - x is loaded first-half-of-T-major, interleaved per-kc-chunk with wq, so
    span-0 projection matmuls start as soon as their kc operands land; a
    short PE warmup stream covers the DMA-arm window (HAM un-throttle). Bulk
    loads ride sync/gpsimd/vector queues -- NEVER the scalar queue, whose
    instruction stream must stay free for the exp activations.
  - Out-projection: pair 0 = spans 0+1 stacked (M=128), spread over span 3's
    kt loop as fillers. Spans 2 and 3 use per-(b,ns) M=64 units: span 2's
    depend only on A2A(2) and fill the span-3 A2A flight; span 3's follow the
    single full-span A2A(3) that fires right after the last normalize. bo is
    folded in on DVE at PSUM evacuation. Output DMAs ride the gpsimd queue so
    they never queue behind gather DMAs.

Dtypes: all matmul operands bf16 with fp32 PSUM accumulation; softmax exp(s)
without row-max (scores O(1), scale folded into Wq host-side).
"""

import os
import numpy as np
import ml_dtypes

BF16 = ml_dtypes.bfloat16

B, T, D, H = 2, 2048, 1024, 16
HD = D // H                     # 64
NCORES = 8
GROUPS = 4                      # cores per batch (tensor-parallel degree)
HL = H // GROUPS                # heads per core = 4
DL = D // GROUPS                # dims per core = 256
SP = 512                        # free-dim span per matmul (one PSUM bank, fp32)
QS = T // SP                    # 4 q spans
KT = T // 128                   # 16 k tiles
QB = 64                         # q columns per rank territory per span
SCALE = HD ** -0.5

_CACHE = {}


def _build_program():
    import concourse.bass as bass  # noqa: F401  (registers bass machinery)
    import concourse.tile as tile
    from concourse import bacc, mybir

    f32 = mybir.dt.float32
    bf16 = mybir.dt.bfloat16
    Exp = mybir.ActivationFunctionType.Exp

    nc = bacc.Bacc("TRN2", target_bir_lowering=False, debug=False,
                   num_devices=NCORES)

    xT = nc.dram_tensor("xT", [D, T], bf16, kind="ExternalInput")
    wqT = nc.dram_tensor("wqT", [D, DL], bf16, kind="ExternalInput")
    wkT = nc.dram_tensor("wkT", [D, DL], bf16, kind="ExternalInput")
    wvT = nc.dram_tensor("wvT", [D, DL], bf16, kind="ExternalInput")
    woT = nc.dram_tensor("woT", [D, D], bf16, kind="ExternalInput")
    bqP = nc.dram_tensor("bqP", [128, 2], f32, kind="ExternalInput")
    bkP = nc.dram_tensor("bkP", [128, 2], f32, kind="ExternalInput")
    bv = nc.dram_tensor("bv", [1, DL], bf16, kind="ExternalInput")
    bo = nc.dram_tensor("bo", [1, D], f32, kind="ExternalInput")
    maskd = nc.dram_tensor("maskd", [128, 128], bf16, kind="ExternalInput")
    onesb = nc.dram_tensor("onesb", [1, 128], bf16, kind="ExternalInput")
    out_ext = nc.dram_tensor("out", [QS, 128, D], f32, kind="ExternalOutput")

    ALL8 = [[0, 1, 2, 3, 4, 5, 6, 7]]

    with tile.TileContext(nc) as tc:
        with tc.tile_pool(name="main", bufs=1) as main, \
             tc.tile_pool(name="dram", bufs=1, space="DRAM") as dram:
            qT_s = main.tile([128, 2, T], bf16)
            kT_s = main.tile([128, 2, T], bf16)
            v_s = main.tile([128, KT, HL * 65], bf16)
            yT_s = main.tile([128, 2, T], bf16)
            woT_s = main.tile([128, 8, D], bf16)
            bq_s = main.tile([128, 2], f32)
            bk_s = main.tile([128, 2], f32)
            bo_bc = main.tile([128, D], f32)
            onesb_s = main.tile([128, 128], bf16)
            bv_bc = main.tile([128, DL], bf16)
            maskd_s = main.tile([128, 128], bf16)
            warm_s = main.tile([128, 2], f32)
            warm_sb = main.tile([128, SP], bf16)

            # per-span A2A staging (separate tiles avoid false DRAM deps)
            a2a_in = [dram.tile([8, DL, QB], bf16, name=f"a2ain{i}")
                      for i in range(QS)]
            a2a_out = [dram.tile([8, DL, QB], bf16, name=f"a2aout{i}")
                       for i in range(QS)]

            # PE warmup: back-to-back matmuls on scratch data while the first
            # input DMAs stream in (HAM un-throttle); real projection matmuls
            # take over as soon as their kc operands land
            nc.vector.memset(warm_sb, 1.0)
            with tc.tile_pool(name="warm_psum", bufs=1,
                              space="PSUM") as warm_psum:
                wps = warm_psum.tile([128, SP], f32, tag="w")
                for _ in range(12):
                    nc.tensor.matmul(wps, lhsT=warm_sb[:, 0:128],
                                     rhs=warm_sb, start=True, stop=True)

            # tiny high-priority loads on the sync queue
            nc.sync.dma_start(out=bq_s, in_=bqP[:])
            nc.sync.dma_start(out=bk_s, in_=bkP[:])
            # pre-load the ACT Exp table (the only set the kernel ever uses)
            nc.scalar.activation(warm_s, warm_sb[:, 0:2], Exp)
            # small loads on the scalar queue
            nc.scalar.dma_start(out=onesb_s,
                                in_=onesb[:].to_broadcast([128, 128]))
            nc.scalar.dma_start(out=bv_bc, in_=bv[:].to_broadcast([128, DL]))
            nc.scalar.dma_start(out=bo_bc, in_=bo[:].to_broadcast([128, D]))
            # ones column at index 64 of each head's 65-wide block of v_aug
            nc.vector.memset(v_s, 1.0)

            # ---------------- input loads ----------------
            xt_s = main.tile([128, 8, T], bf16)
            wq_s = main.tile([128, 8, DL], bf16)
            wk_s = main.tile([128, 8, DL], bf16)
            wv_s = main.tile([128, 8, DL], bf16)

            wq_r = wqT[:].rearrange("(c p) n -> c p n", p=128)
            wk_r = wkT[:].rearrange("(c p) n -> c p n", p=128)
            wv_r = wvT[:].rearrange("(c p) n -> c p n", p=128)
            xT_r = xT[:].rearrange("(c p) t -> c p t", p=128)
            woT_r = woT[:].rearrange("(c p) n -> c p n", p=128)
            # lead: x first half (spans 0-1, 2KB/partition packets)
            # interleaved per-chunk with wq so projection kc operands land
            # progressively; round-robin over sync/gpsimd/scalar
            lead = []
            for c in range(3):
                lead.append((xt_s[:, c, 0:2 * SP], xT_r[c][:, 0:2 * SP]))
            for c in range(3):
                lead.append((wq_s[:, c, :], wq_r[c]))
            for c in range(3, 8):
                lead.append((xt_s[:, c, 0:2 * SP], xT_r[c][:, 0:2 * SP]))
                lead.append((wq_s[:, c, :], wq_r[c]))
            q3 = [nc.sync, nc.gpsimd, nc.scalar]
            for i, (dst, src) in enumerate(lead):
                q3[i % 3].dma_start(out=dst, in_=src)
            # mid: wk/wv/maskd on sync+gpsimd (scalar queue must stay free
            # for the exp stream from here on)
            mid = [(wk_s[:, c, :], wk_r[c]) for c in range(8)]
            mid += [(wv_s[:, c, :], wv_r[c]) for c in range(8)]
            mid.append((maskd_s, maskd[:]))
            q2 = [nc.sync, nc.gpsimd]
            for i, (dst, src) in enumerate(mid):
                q2[i % 2].dma_start(out=dst, in_=src)
            # tail loads: x second half + woT on sync/gpsimd (scalar's
            # instruction stream stays free for the exp activations)
            tl = [(xt_s[:, c, 2 * SP:T], xT_r[c][:, 2 * SP:T])
                  for c in range(8)]
            tl += [(woT_s[:, c, :], woT_r[c]) for c in range(8)]
            for i, (dst, src) in enumerate(tl):
                q2[i % 2].dma_start(out=dst, in_=src)

            # ---- single pipelined phase ----
            with tc.tile_pool(name="attn_t", bufs=4) as attn_t, \
                 tc.tile_pool(name="nrm", bufs=2) as nrm, \
                 tc.tile_pool(name="op_sb", bufs=4) as op_sb, \
                 tc.tile_pool(name="sc_psum", bufs=2, space="PSUM") as sc_psum, \
                 tc.tile_pool(name="av_psum", bufs=2, space="PSUM") as av_psum, \
                 tc.tile_pool(name="op_psum", bufs=2, space="PSUM") as op_psum:

                def attention_span(qs, fillers, self_norm=False):
                    # denominator rows at partitions 0/32/64/96 (engine APs
                    # must start 32-aligned); memset keeps unused rows finite
                    den_stack = nrm.tile([97, SP], f32, tag="den")
                    nc.vector.memset(den_stack, 1.0)
                    rec32 = nrm.tile([97, SP], f32, tag="rec32")
                    rec_bf = nrm.tile([97, SP], bf16, tag="recf")
                    nkt = 4 * qs + 4  # causal: later k tiles are all-masked
                    span = slice(qs * SP, (qs + 1) * SP)
                    # carry pre-load: pop ~2 fillers in the first steps of
                    # the span, where the PE would otherwise micro-idle
                    # waiting on exp(0) and HAM would re-throttle
                    pace = {"left": 2 * nkt, "carry": 2.0}

                    def pop_fillers():
                        frac = pace["carry"] + len(fillers) / pace["left"]
                        n_pop = int(frac)
                        pace["carry"] = frac - n_pop
                        pace["left"] -= 1
                        for _ in range(min(n_pop, len(fillers))):
                            fillers.pop(0)()

                    for p in range(2):  # head pair = mc chunk p
                        qa = qT_s[0:64, p, span]
                        qb = qT_s[64:128, p, span]
                        ya = av_psum.tile([65, SP], f32, tag="av")
                        yb = av_psum.tile([65, SP], f32, tag="av")

                        def sc_pair(kt):
                            # diagonal tiles: q columns < 128*(kt-4qs) are
                            # fully masked; stream only the valid sub-range
                            j0 = max(0, (kt - 4 * qs) * 128)
                            scp = sc_psum.tile([128, 2 * SP], f32, tag="sc")
                            nc.tensor.matmul(
                                scp[:, j0:SP],
                                lhsT=kT_s[0:64, p, kt * 128:(kt + 1) * 128],
                                rhs=qa[:, j0:SP], start=True, stop=True)
                            nc.tensor.matmul(
                                scp[:, SP + j0:2 * SP],
                                lhsT=kT_s[64:128, p, kt * 128:(kt + 1) * 128],
                                rhs=qb[:, j0:SP], start=True, stop=True)
                            return scp

                        # software pipeline: scores kt+1 are emitted before
                        # the AV matmuls of kt so the in-order PE queue keeps
                        # feeding ACT while AV waits on exp kt
                        scp = sc_pair(0)
                        for kt in range(nkt):
                            atp = attn_t.tile([128, 2 * SP], bf16, tag="at")
                            j0e = max(0, (kt - 4 * qs) * 128)
                            if j0e >= 256:
                                # mostly-masked diagonal tile: two narrow
                                # exps over the valid ranges beat one full-
                                # width one
                                nc.scalar.activation(atp[:, j0e:SP],
                                                     scp[:, j0e:SP], Exp)
                                nc.scalar.activation(
                                    atp[:, SP + j0e:2 * SP],
                                    scp[:, SP + j0e:2 * SP], Exp)
                            else:
                                nc.scalar.activation(atp, scp, Exp)
                            if kt >= 4 * qs:
                                # diagonal tile: only its 128-col diagonal
                                # block needs masking and that block is the
                                # same tril(128) for every tile
                                jm = (kt - 4 * qs) * 128
                                nc.vector.tensor_mul(
                                    atp[:, jm:jm + 128],
                                    atp[:, jm:jm + 128], maskd_s)
                                nc.vector.tensor_mul(
                                    atp[:, SP + jm:SP + jm + 128],
                                    atp[:, SP + jm:SP + jm + 128], maskd_s)
                            if kt + 1 < nkt:
                                scp = sc_pair(kt + 1)
                            # independent fillers run while ACT produces
                            # exp(kt); they must precede the exp-gated AVs
                            pop_fillers()
                            j0 = max(0, (kt - 4 * qs) * 128)
                            nc.tensor.matmul(
                                ya[:, j0:SP],
                                lhsT=v_s[:, kt, (2 * p) * 65:
                                             (2 * p + 1) * 65],
                                rhs=atp[:, j0:SP],
                                start=(kt == 0), stop=(kt == nkt - 1))
                            nc.tensor.matmul(
                                yb[:, j0:SP],
                                lhsT=v_s[:, kt, (2 * p + 1) * 65:
                                             (2 * p + 2) * 65],
                                rhs=atp[:, SP + j0:2 * SP],
                                start=(kt == 0), stop=(kt == nkt - 1))
                        # evacuate unnormalized yT' + denominators so the
                        # PSUM banks free for the next pair; the last pair's
                        # evacuation is on the A2A(3)-trigger critical path,
                        # so split it across DVE and the idle ACT engine
                        if self_norm and p == 1:
                            nc.vector.tensor_copy(yT_s[0:64, p, span],
                                                  ya[0:64, :])
                            nc.scalar.copy(yT_s[64:128, p, span],
                                           yb[0:64, :])
                            nc.scalar.copy(
                                den_stack[64 * p:64 * p + 1, :], ya[64:65, :])
                            nc.scalar.copy(
                                den_stack[64 * p + 32:64 * p + 33, :],
                                yb[64:65, :])
                        else:
                            nc.vector.tensor_copy(yT_s[0:64, p, span],
                                                  ya[0:64, :])
                            nc.vector.tensor_copy(yT_s[64:128, p, span],
                                                  yb[0:64, :])
                            nc.vector.tensor_copy(
                                den_stack[64 * p:64 * p + 1, :], ya[64:65, :])
                            nc.vector.tensor_copy(
                                den_stack[64 * p + 32:64 * p + 33, :],
                                yb[64:65, :])
                        if self_norm:
                            # last span: full-width DVE reciprocal per pair
                            # (the custom DVE op wants partition offset 0;
                            # pair-1 rows are memset 1.0 until valid and the
                            # second pass overwrites everything)
                            nc.vector.reciprocal_approx_fast(
                                out=rec32, in_=den_stack)
                            nc.vector.tensor_copy(rec_bf, rec32)
                            if p == 0:
                                # front-insert: the mc0 normalize must fire
                                # early in pair 1's sweep
                                fillers[0:0] = [norm_h(qs, rec_bf, 0),
                                                norm_h(qs, rec_bf, 1)]
                                pace["carry"] += 2.0
                    if not self_norm:
                        # single-instruction DVE reciprocal: no ACT table
                        # switches, no span-boundary exp contention
                        nc.vector.reciprocal_approx_fast(out=rec32,
                                                         in_=den_stack)
                        nc.vector.tensor_copy(rec_bf, rec32)
                    return rec_bf

                def norm_h(qs, rec_bf, h):
                    def f():
                        span = slice(qs * SP, (qs + 1) * SP)
                        mc, r0 = divmod(h, 2)
                        r0 *= 64
                        rb = op_psum.tile([64, SP], f32, tag="op")
                        r0p = 32 * h
                        nc.tensor.matmul(rb,
                                         lhsT=onesb_s[r0p:r0p + 1, 0:64],
                                         rhs=rec_bf[r0p:r0p + 1, :],
                                         start=True, stop=True,
                                         tile_position=(r0p, 0))
                        nc.vector.tensor_mul(yT_s[r0:r0 + 64, mc, span],
                                             yT_s[r0:r0 + 64, mc, span],
                                             rb)
                    return f

                def stage_a2a(qs):
                    def f():
                        span = slice(qs * SP, (qs + 1) * SP)
                        in_r = a2a_in[qs][:].rearrange(
                            "j (two p) q -> two p j q", p=128)
                        for mc in range(2):
                            nc.sync.dma_start(
                                out=in_r[mc],
                                in_=yT_s[:, mc, span].rearrange(
                                    "p (j q) -> p j q", q=QB))
                        nc.gpsimd.collective_compute(
                            "AllToAll", mybir.AluOpType.bypass,
                            replica_groups=ALL8,
                            ins=[a2a_in[qs][:].opt()],
                            outs=[a2a_out[qs][:].opt()])
                    return f

                def proj_qk(w_s, b_s, dst, mc, s):
                    # qT/kT group: out[dims-chunk mc, t-span s]; bias added
                    # on DVE during the PSUM->SBUF evacuation
                    def f():
                        ps = op_psum.tile([128, SP], f32, tag="op",
                                          name="pj")
                        for kc in range(8):
                            nc.tensor.matmul(
                                ps,
                                lhsT=w_s[:, kc, mc * 128:(mc + 1) * 128],
                                rhs=xt_s[:, kc, s * SP:(s + 1) * SP],
                                start=(kc == 0), stop=(kc == 7))
                        nc.vector.tensor_scalar_add(
                            dst[:, mc, s * SP:(s + 1) * SP],
                            ps, b_s[:, mc:mc + 1])
                    return f

                def proj_v(mt):
                    # v tile in natural [t, d] layout; bias via DVE add into
                    # the 65-stride v_aug slots
                    def f():
                        ps = op_psum.tile([128, SP], f32, tag="op",
                                          name="pjv")
                        for kc in range(8):
                            nc.tensor.matmul(
                                ps[:, 0:DL],
                                lhsT=xt_s[:, kc, mt * 128:(mt + 1) * 128],
                                rhs=wv_s[:, kc, :],
                                start=(kc == 0), stop=(kc == 7))
                        nc.vector.tensor_add(
                            v_s[:, mt, :].rearrange(
                                "p (h d) -> p h d", d=65)[:, :, 0:64],
                            ps[:, 0:DL].rearrange("p (h d) -> p h d", d=64),
                            bv_bc.rearrange("p (h d) -> p h d", d=64))
                    return f

                def proj_span_fillers(s):
                    fs = []
                    for mc in range(2):
                        fs.append(proj_qk(wq_s, bq_s, qT_s, mc, s))
                    for mc in range(2):
                        fs.append(proj_qk(wk_s, bk_s, kT_s, mc, s))
                    for mt in range(4 * s, 4 * s + 4):
                        fs.append(proj_v(mt))
                    return fs

                def outproj_pair0_fillers():
                    """Out-projection for spans (0,1) with M=128 (both spans'
                    64-q territories stacked), split into per-(b,ns) filler
                    chunks; spread over span 3's kt loop."""
                    yg = op_sb.tile([128, 8, 2, 2 * QB], bf16, tag="yg",
                                    bufs=1)

                    def gather(sp):
                        def f():
                            out_r = a2a_out[sp][:].rearrange(
                                "(b j2) (h p) q -> b p (j2 h) q", j2=4, p=128)
                            m = sp % 2
                            for b in range(2):
                                nc.sync.dma_start(
                                    out=yg[:, :, b, m * QB:(m + 1) * QB],
                                    in_=out_r[b])
                        return f

                    def po_mms(b, ns, kcs, box, last=False):
                        def f():
                            if not box:
                                box.append(op_psum.tile(
                                    [128, SP], f32, tag="op", name="po"))
                            po = box[0]
                            for kc in kcs:
                                nc.tensor.matmul(
                                    po, lhsT=yg[:, kc, b, :],
                                    rhs=woT_s[:, kc, ns * SP:(ns + 1) * SP],
                                    start=(kc == 0),
                                    stop=(last and kc == kcs[-1]))
                            if last:
                                ob = op_sb.tile([128, SP], f32, tag="ob")
                                nc.vector.tensor_add(
                                    ob, po, bo_bc[:, ns * SP:(ns + 1) * SP])
                                for m in range(2):
                                    nc.gpsimd.dma_start(
                                        out=out_ext[m, b * 64:(b + 1) * 64,
                                                    ns * SP:(ns + 1) * SP],
                                        in_=ob[m * 64:(m + 1) * 64, :])
                        return f

                    fs = [gather(0), gather(1)]
                    for b in range(2):
                        for ns in range(2):
                            box = []
                            fs.append(po_mms(b, ns, [0, 1, 2], box))
                            fs.append(po_mms(b, ns, [3, 4, 5], box))
                            fs.append(po_mms(b, ns, [6, 7], box, last=True))
                    return fs

                def gather_span(sp, tag):
                    yg = op_sb.tile([128, 8, 2, QB], bf16, tag=tag, bufs=1,
                                    name=tag)
                    out_r = a2a_out[sp][:].rearrange(
                        "(b j2) (h p) q -> b p (j2 h) q", j2=4, p=128)
                    for b in range(2):
                        nc.sync.dma_start(out=yg[:, :, b, :], in_=out_r[b])
                    return yg

                def po_unit(sp, yg, b, ns, alt=False):
                    # M=64 out-projection of span sp's territory; the final
                    # units alternate evac/DMA engines so the teardown
                    # doesn't serialize on DVE + one DMA queue
                    def f():
                        po = op_psum.tile([64, SP], f32, tag="op", name="pou")
                        for kc in range(8):
                            nc.tensor.matmul(
                                po, lhsT=yg[:, kc, b, :],
                                rhs=woT_s[:, kc, ns * SP:(ns + 1) * SP],
                                start=(kc == 0), stop=(kc == 7))
                        ob = op_sb.tile([64, SP], f32, tag="ob")
                        nc.vector.tensor_add(
                            ob, po, bo_bc[0:64, ns * SP:(ns + 1) * SP])
                        dq = nc.sync if alt else nc.gpsimd
                        dq.dma_start(
                            out=out_ext[sp, b * 64:(b + 1) * 64,
                                        ns * SP:(ns + 1) * SP],
                            in_=ob)
                    return f

                # prologue: just enough of span 0's projections to start its
                # attention; the rest ride the kt loop as early fillers
                p0 = proj_span_fillers(0)
                for f in [p0[0], p0[2], p0[4], p0[5]]:  # q0, k0, v0, v1
                    f()
                pend = [p0[6], p0[7], p0[1], p0[3]]     # v2, v3, q1, k1
                rec = {}
                for qs in range(QS):
                    if qs < 3:
                        # span qs+1's projections drain during span qs
                        pend.extend(proj_span_fillers(qs + 1))
                    else:
                        # A2A(0)/(1) completed spans ago: pair-0's b=0
                        # out-projection spreads over span 3's kt loop; the
                        # b=1 units are deferred into the A2A(3) flight
                        opf = outproj_pair0_fillers()
                        pend.extend(opf[:8])
                    rec[qs] = attention_span(qs, pend, self_norm=(qs == 3))
                    if qs < 3:
                        nf = [norm_h(qs, rec[qs], h) for h in range(HL)]
                        if qs == 2:
                            # norms/A2A(2) pop first in span 3 so the A2A(2)
                            # flight fully precedes its po consumers
                            pend[0:0] = nf + [stage_a2a(qs)]
                        else:
                            pend.extend(nf + [stage_a2a(qs)])
                for f in pend:
                    f()

                # ---- tail: normalize span-3 mc1, fire the single full-span
                # A2A(3), then fill its ~20us flight with span-2's
                # out-projection, pair-0's deferred b=1 units, and a short
                # dummy-matmul bridge (keeps HAM at full clock so the
                # A2A-gated span-3 out-projection runs warm).
                norm_h(3, rec[3], 2)()
                norm_h(3, rec[3], 3)()
                stage_a2a(3)()
                yg2 = gather_span(2, "yg2")
                for b in range(2):
                    for ns in range(2):
                        po_unit(2, yg2, b, ns)()
                for f in opf[8:]:
                    f()
                dps = sc_psum.tile([128, SP], f32, tag="sc", name="dummy")
                for _ in range(16):
                    nc.tensor.matmul(dps, lhsT=warm_sb[:, 0:128],
                                     rhs=warm_sb, start=True, stop=True)
                yg3 = gather_span(3, "yg3")
                for i, (b, ns) in enumerate(((0, 0), (0, 1), (1, 0), (1, 1))):
                    po_unit(3, yg3, b, ns, alt=(i % 2 == 1))()

    nc.compile()
    return nc


def _get_program():
    if "nc" not in _CACHE:
        _CACHE["nc"] = _build_program()
    return _CACHE["nc"]


def _make_in_maps(x, mask, Wq, bq, Wk, bk, Wv, bv, Wo, bo):
    x = np.asarray(x, np.float32)
    mask = np.asarray(mask, bool)
    Wq = np.asarray(Wq, np.float32)
    Wk = np.asarray(Wk, np.float32)
    Wv = np.asarray(Wv, np.float32)
    Wo = np.asarray(Wo, np.float32)
    bq = np.asarray(bq, np.float32)
    bk = np.asarray(bk, np.float32)
    bv = np.asarray(bv, np.float32)
    bo = np.asarray(bo, np.float32)

    woT = np.ascontiguousarray(Wo.T).astype(BF16)
    in_maps = []
    per_batch = {}
    for b in range(B):
        xTb = np.ascontiguousarray(x[b].T)
        # the only masking the kernel applies is the 128x128 diagonal
        # block (identical for every diagonal tile of a causal mask)
        md = mask[b, 0].T[0:128, 0:128].astype(np.float32)
        per_batch[b] = (xTb, md)
    for c in range(NCORES):
        b, g = divmod(c, GROUPS)
        sl = slice(g * DL, (g + 1) * DL)
        xTb, md = per_batch[b]
        in_maps.append({
            "xT": xTb.astype(BF16),
            "wqT": np.ascontiguousarray((Wq[sl] * SCALE).T).astype(BF16),
            "wkT": np.ascontiguousarray(Wk[sl].T).astype(BF16),
            "wvT": np.ascontiguousarray(Wv[sl].T).astype(BF16),
            "woT": woT,
            "bqP": np.ascontiguousarray((bq[sl] * SCALE).reshape(2, 128).T),
            "bkP": np.ascontiguousarray(bk[sl].reshape(2, 128).T),
            "bv": bv[sl].reshape(1, DL).astype(BF16),
            "bo": bo.reshape(1, D).astype(np.float32),
            "maskd": md.astype(BF16),
            "onesb": np.ones((1, 128), BF16),
        })
    return in_maps


def _capture_profile(nc, in_maps, tmpdir):
    """Run with NTFF capture and process the profile ourselves. Returns
    (results, exec_time_ns|None)."""
    import glob
    import json
    import re
    import subprocess
    from trn_agent_boot.trn_boot import _ntff_profile_via_ctypes
    from concourse import bass2jax

    hook = _ntff_profile_via_ctypes("/opt/axon/libaxon_pjrt.so")
    if hook is None:
        raise RuntimeError("libaxon_pjrt.so lacks NTFF profile symbols")
    os.makedirs(tmpdir, exist_ok=True)
    with hook(tmpdir, [0]):
        results = bass2jax.run_bass_via_pjrt(nc, in_maps, n_cores=NCORES)

    ntffs = glob.glob(os.path.join(tmpdir, "*_body*-device*.ntff"))
    best = None
    for f in ntffs:
        if re.search(r"executable(\d+)-device000000", f):
            if best is None or os.path.getmtime(f) > os.path.getmtime(best):
                best = f
    if best is None:
        raise RuntimeError(f"no NTFF produced in {tmpdir}")
    neff = re.sub(r"-device\d+-execution-\d+\.ntff$", ".neff", best)
    out_json = os.path.join(tmpdir, "prof.json")
    subprocess.check_call(
        ["neuron-profile", "view", "--ignore-nc-buf-usage", "-s", best,
         "-n", neff, "--output-format=json", f"--output-file={out_json}"],
        cwd=tmpdir)
    summary = json.load(open(out_json))["summary"][0]
    return results, int(summary["total_time"] * 1e9)


def kernel(x, mask, Wq, bq, Wk, bk, Wv, bv, Wo, bo):
    from concourse import bass_utils

    in_maps = _make_in_maps(x, mask, Wq, bq, Wk, bk, Wv, bv, Wo, bo)
    nc = _get_program()

    trace = bool(int(os.environ.get("MHA_TRACE", "0")))
    tmpdir = os.environ.get("MHA_TRACE_DIR") or None
    results = None
    if trace and tmpdir:
        try:
            results, exec_ns = _capture_profile(nc, in_maps, tmpdir)
            _CACHE["last_exec_time_ns"] = exec_ns
        except Exception as e:  # profiling is best-effort
            print(f"profiling unavailable: {type(e).__name__}: {e}")
            results = None
    if results is None:
        results = bass_utils.run_bass_kernel_spmd(
            nc, in_maps, core_ids=list(range(NCORES))).results
        _CACHE.setdefault("last_exec_time_ns", None)

    # core c's out[qs] holds rows (q = qs*512 + c*64 + i) for batch 0
    # (rows 0-63) and batch 1 (rows 64-127)
    out = np.empty((B, T, D), np.float32)
    for c in range(NCORES):
        o = results[c]["out"]
        for qs in range(QS):
            q0 = qs * SP + c * QB
            out[0, q0:q0 + QB] = o[qs, 0:QB]
            out[1, q0:q0 + QB] = o[qs, QB:2 * QB]
    return out


# revision 16
# speedup vs baseline: 1.0665x; 1.0165x over previous
"""Causal multi-head attention (B=2, T=2048, D=1024, H=16) on 8 TRN2 NeuronCores.

Sharding: core c = (batch b = c//4, head-group g = c%4). Each core owns 4 heads
(= 256 contiguous dims of D) of one batch: Megatron-style tensor parallelism on
heads x data parallelism on batch.

Design (single fully-pipelined phase; everything but span 0's attention is
emitted as "fillers" interleaved into the attention kt loops so the in-order
PE queue has no phase boundaries and never head-of-line blocks):
  - Out-projection reduction via per-q-span 8-way bf16 AllToAll of the
    normalized attention output yT (rank r's territory = 64-col q-block r of
    each 512-q span, for BOTH batches -- SPMD-uniform, no junk shards). Each
    core then out-projects its territory with the full Wo.
  - Score matmuls pack the two heads of an mc-chunk as two concurrent K=64
    row-group tiles into one [128, 2*512] 2-bank PSUM tile; ONE exp per pair.
    The kt loop software-pipelines scores kt+1 ahead of the AV matmuls of kt;
    fillers pop BEFORE the exp-gated AV matmuls so the in-order PE queue keeps
    running independent work while ACT produces exp(kt). ACT runs ONLY exp --
    a single table set, loaded once at warmup, zero mid-run table switches.
  - AV uses the v_aug 65th-column trick (denominator accumulates as row 64);
    1/den via the single-instruction DVE approx reciprocal (~18 bits, plenty
    upstream of bf16); normalization = PE rank-1 broadcast of 1/den +
    in-place DVE multiply, one span behind attention.
  - x is loaded first-half-of-T-major, interleaved per-kc-chunk with wq, so
    span-0 projection matmuls start as soon as their kc operands land; a
    short PE warmup stream covers the DMA-arm window (HAM un-throttle). Bulk
    loads ride sync/gpsimd/vector queues -- NEVER the scalar queue, whose
    instruction stream must stay free for the exp activations.
  - Out-projection: pair 0 = spans 0+1 stacked (M=128), spread over span 3's
    kt loop as fillers. Spans 2 and 3 use per-(b,ns) M=64 units: span 2's
    depend only on A2A(2) and fill the span-3 A2A flight; span 3's follow the
    single full-span A2A(3) that fires right after the last normalize. bo is
    folded in on DVE at PSUM evacuation. Output DMAs ride the gpsimd queue so
    they never queue behind gather DMAs.

Dtypes: all matmul operands bf16 with fp32 PSUM accumulation; softmax exp(s)
without row-max (scores O(1), scale folded into Wq host-side).
"""

import os
import numpy as np
import ml_dtypes

BF16 = ml_dtypes.bfloat16

B, T, D, H = 2, 2048, 1024, 16
HD = D // H                     # 64
NCORES = 8
GROUPS = 4                      # cores per batch (tensor-parallel degree)
HL = H // GROUPS                # heads per core = 4
DL = D // GROUPS                # dims per core = 256
SP = 512                        # free-dim span per matmul (one PSUM bank, fp32)
QS = T // SP                    # 4 q spans
KT = T // 128                   # 16 k tiles
QB = 64                         # q columns per rank territory per span
SCALE = HD ** -0.5

_CACHE = {}


def _build_program():
    import concourse.bass as bass  # noqa: F401  (registers bass machinery)
    import concourse.tile as tile
    from concourse import bacc, mybir

    f32 = mybir.dt.float32
    bf16 = mybir.dt.bfloat16
    Exp = mybir.ActivationFunctionType.Exp

    nc = bacc.Bacc("TRN2", target_bir_lowering=False, debug=False,
                   num_devices=NCORES)

    xT = nc.dram_tensor("xT", [D, T], bf16, kind="ExternalInput")
    wqT = nc.dram_tensor("wqT", [D, DL], bf16, kind="ExternalInput")
    wkT = nc.dram_tensor("wkT", [D, DL], bf16, kind="ExternalInput")
    wvT = nc.dram_tensor("wvT", [D, DL], bf16, kind="ExternalInput")
    woT = nc.dram_tensor("woT", [D, D], bf16, kind="ExternalInput")
    bqP = nc.dram_tensor("bqP", [128, 2], f32, kind="ExternalInput")
    bkP = nc.dram_tensor("bkP", [128, 2], f32, kind="ExternalInput")
    bv = nc.dram_tensor("bv", [1, DL], bf16, kind="ExternalInput")
    bo = nc.dram_tensor("bo", [1, D], f32, kind="ExternalInput")
    maskd = nc.dram_tensor("maskd", [128, 128], bf16, kind="ExternalInput")
    onesb = nc.dram_tensor("onesb", [1, 128], bf16, kind="ExternalInput")
    out_ext = nc.dram_tensor("out", [QS, 128, D], f32, kind="ExternalOutput")

    ALL8 = [[0, 1, 2, 3, 4, 5, 6, 7]]

    with tile.TileContext(nc) as tc:
        with tc.tile_pool(name="main", bufs=1) as main, \
             tc.tile_pool(name="dram", bufs=1, space="DRAM") as dram:
            qT_s = main.tile([128, 2, T], bf16)
            kT_s = main.tile([128, 2, T], bf16)
            v_s = main.tile([128, KT, HL * 65], bf16)
            yT_s = main.tile([128, 2, T], bf16)
            woT_s = main.tile([128, 8, D], bf16)
            bq_s = main.tile([128, 2], f32)
            bk_s = main.tile([128, 2], f32)
            bo_bc = main.tile([128, D], f32)
            onesb_s = main.tile([128, 128], bf16)
            bv_bc = main.tile([128, DL], bf16)
            maskd_s = main.tile([128, 128], bf16)
            warm_s = main.tile([128, 2], f32)
            warm_sb = main.tile([128, SP], bf16)

            # per-span A2A staging (separate tiles avoid false DRAM deps)
            a2a_in = [dram.tile([8, DL, QB], bf16, name=f"a2ain{i}")
                      for i in range(QS)]
            a2a_out = [dram.tile([8, DL, QB], bf16, name=f"a2aout{i}")
                       for i in range(QS)]

            # PE warmup: back-to-back matmuls on scratch data while the first
            # input DMAs stream in (HAM un-throttle); real projection matmuls
            # take over as soon as their kc operands land
            nc.vector.memset(warm_sb, 1.0)
            with tc.tile_pool(name="warm_psum", bufs=1,
                              space="PSUM") as warm_psum:
                wps = warm_psum.tile([128, SP], f32, tag="w")
                for _ in range(24):
                    nc.tensor.matmul(wps, lhsT=warm_sb[:, 0:128],
                                     rhs=warm_sb, start=True, stop=True)

            # tiny high-priority loads on the sync queue
            nc.sync.dma_start(out=bq_s, in_=bqP[:])
            nc.sync.dma_start(out=bk_s, in_=bkP[:])
            # pre-load the ACT Exp table (the only set the kernel ever uses)
            nc.scalar.activation(warm_s, warm_sb[:, 0:2], Exp)
            # small loads on the scalar queue
            nc.scalar.dma_start(out=onesb_s,
                                in_=onesb[:].to_broadcast([128, 128]))
            nc.scalar.dma_start(out=bv_bc, in_=bv[:].to_broadcast([128, DL]))
            nc.scalar.dma_start(out=bo_bc, in_=bo[:].to_broadcast([128, D]))
            # ones column at index 64 of each head's 65-wide block of v_aug
            nc.vector.memset(v_s, 1.0)

            # ---------------- input loads ----------------
            xt_s = main.tile([128, 8, T], bf16)
            wq_s = main.tile([128, 8, DL], bf16)
            wk_s = main.tile([128, 8, DL], bf16)
            wv_s = main.tile([128, 8, DL], bf16)

            wq_r = wqT[:].rearrange("(c p) n -> c p n", p=128)
            wk_r = wkT[:].rearrange("(c p) n -> c p n", p=128)
            wv_r = wvT[:].rearrange("(c p) n -> c p n", p=128)
            xT_r = xT[:].rearrange("(c p) t -> c p t", p=128)
            woT_r = woT[:].rearrange("(c p) n -> c p n", p=128)
            # lead: x first half (spans 0-1, 2KB/partition packets)
            # interleaved per-chunk with wq so projection kc operands land
            # progressively; round-robin over sync/gpsimd/scalar
            lead = []
            for c in range(3):
                lead.append((xt_s[:, c, 0:2 * SP], xT_r[c][:, 0:2 * SP]))
            for c in range(3):
                lead.append((wq_s[:, c, :], wq_r[c]))
            for c in range(3, 8):
                lead.append((xt_s[:, c, 0:2 * SP], xT_r[c][:, 0:2 * SP]))
                lead.append((wq_s[:, c, :], wq_r[c]))
            q3 = [nc.sync, nc.gpsimd, nc.scalar]
            for i, (dst, src) in enumerate(lead):
                q3[i % 3].dma_start(out=dst, in_=src)
            # mid: wk/wv/maskd on sync+gpsimd (scalar queue must stay free
            # for the exp stream from here on)
            mid = [(wk_s[:, c, :], wk_r[c]) for c in range(8)]
            mid += [(wv_s[:, c, :], wv_r[c]) for c in range(8)]
            mid.append((maskd_s, maskd[:]))
            q2 = [nc.sync, nc.gpsimd]
            for i, (dst, src) in enumerate(mid):
                q2[i % 2].dma_start(out=dst, in_=src)
            # tail loads: x second half + woT on sync/gpsimd (scalar's
            # instruction stream stays free for the exp activations)
            tl = [(xt_s[:, c, 2 * SP:T], xT_r[c][:, 2 * SP:T])
                  for c in range(8)]
            tl += [(woT_s[:, c, :], woT_r[c]) for c in range(8)]
            for i, (dst, src) in enumerate(tl):
                q2[i % 2].dma_start(out=dst, in_=src)

            # ---- single pipelined phase ----
            with tc.tile_pool(name="attn_t", bufs=4) as attn_t, \
                 tc.tile_pool(name="nrm", bufs=2) as nrm, \
                 tc.tile_pool(name="op_sb", bufs=4) as op_sb, \
                 tc.tile_pool(name="sc_psum", bufs=2, space="PSUM") as sc_psum, \
                 tc.tile_pool(name="av_psum", bufs=2, space="PSUM") as av_psum, \
                 tc.tile_pool(name="op_psum", bufs=2, space="PSUM") as op_psum:

                def attention_span(qs, fillers, self_norm=False):
                    # denominator rows at partitions 0/32/64/96 (engine APs
                    # must start 32-aligned); memset keeps unused rows finite
                    den_stack = nrm.tile([97, SP], f32, tag="den")
                    nc.vector.memset(den_stack, 1.0)
                    rec32 = nrm.tile([97, SP], f32, tag="rec32")
                    rec_bf = nrm.tile([97, SP], bf16, tag="recf")
                    nkt = 4 * qs + 4  # causal: later k tiles are all-masked
                    span = slice(qs * SP, (qs + 1) * SP)
                    # carry pre-load: pop ~2 fillers in the first steps of
                    # the span, where the PE would otherwise micro-idle
                    # waiting on exp(0) and HAM would re-throttle
                    pace = {"left": 2 * nkt, "carry": 2.0}

                    def pop_fillers():
                        frac = pace["carry"] + len(fillers) / pace["left"]
                        n_pop = int(frac)
                        pace["carry"] = frac - n_pop
                        pace["left"] -= 1
                        for _ in range(min(n_pop, len(fillers))):
                            fillers.pop(0)()

                    for p in range(2):  # head pair = mc chunk p
                        qa = qT_s[0:64, p, span]
                        qb = qT_s[64:128, p, span]
                        ya = av_psum.tile([65, SP], f32, tag="av")
                        yb = av_psum.tile([65, SP], f32, tag="av")

                        def sc_pair(kt):
                            # diagonal tiles: q columns < 128*(kt-4qs) are
                            # fully masked; stream only the valid sub-range
                            j0 = max(0, (kt - 4 * qs) * 128)
                            scp = sc_psum.tile([128, 2 * SP], f32, tag="sc")
                            nc.tensor.matmul(
                                scp[:, j0:SP],
                                lhsT=kT_s[0:64, p, kt * 128:(kt + 1) * 128],
                                rhs=qa[:, j0:SP], start=True, stop=True)
                            nc.tensor.matmul(
                                scp[:, SP + j0:2 * SP],
                                lhsT=kT_s[64:128, p, kt * 128:(kt + 1) * 128],
                                rhs=qb[:, j0:SP], start=True, stop=True)
                            return scp

                        # software pipeline: scores kt+1 are emitted before
                        # the AV matmuls of kt so the in-order PE queue keeps
                        # feeding ACT while AV waits on exp kt
                        scp = sc_pair(0)
                        for kt in range(nkt):
                            atp = attn_t.tile([128, 2 * SP], bf16, tag="at")
                            j0e = max(0, (kt - 4 * qs) * 128)
                            if j0e >= 256:
                                # mostly-masked diagonal tile: two narrow
                                # exps over the valid ranges beat one full-
                                # width one
                                nc.scalar.activation(atp[:, j0e:SP],
                                                     scp[:, j0e:SP], Exp)
                                nc.scalar.activation(
                                    atp[:, SP + j0e:2 * SP],
                                    scp[:, SP + j0e:2 * SP], Exp)
                            else:
                                nc.scalar.activation(atp, scp, Exp)
                            if kt >= 4 * qs:
                                # diagonal tile: only its 128-col diagonal
                                # block needs masking and that block is the
                                # same tril(128) for every tile
                                jm = (kt - 4 * qs) * 128
                                nc.vector.tensor_mul(
                                    atp[:, jm:jm + 128],
                                    atp[:, jm:jm + 128], maskd_s)
                                nc.vector.tensor_mul(
                                    atp[:, SP + jm:SP + jm + 128],
                                    atp[:, SP + jm:SP + jm + 128], maskd_s)
                            if kt + 1 < nkt:
                                scp = sc_pair(kt + 1)
                            # independent fillers run while ACT produces
                            # exp(kt); they must precede the exp-gated AVs
                            pop_fillers()
                            j0 = max(0, (kt - 4 * qs) * 128)
                            nc.tensor.matmul(
                                ya[:, j0:SP],
                                lhsT=v_s[:, kt, (2 * p) * 65:
                                             (2 * p + 1) * 65],
                                rhs=atp[:, j0:SP],
                                start=(kt == 0), stop=(kt == nkt - 1))
                            nc.tensor.matmul(
                                yb[:, j0:SP],
                                lhsT=v_s[:, kt, (2 * p + 1) * 65:
                                             (2 * p + 2) * 65],
                                rhs=atp[:, SP + j0:2 * SP],
                                start=(kt == 0), stop=(kt == nkt - 1))
                        # evacuate unnormalized yT' + denominators so the
                        # PSUM banks free for the next pair; the last pair's
                        # evacuation is on the A2A(3)-trigger critical path,
                        # so split it across DVE and the idle ACT engine
                        if self_norm and p == 1:
                            nc.vector.tensor_copy(yT_s[0:64, p, span],
                                                  ya[0:64, :])
                            nc.scalar.copy(yT_s[64:128, p, span],
                                           yb[0:64, :])
                            nc.scalar.copy(
                                den_stack[64 * p:64 * p + 1, :], ya[64:65, :])
                            nc.scalar.copy(
                                den_stack[64 * p + 32:64 * p + 33, :],
                                yb[64:65, :])
                        else:
                            # ya's copies first: the next pair's first AV
                            # reuses ya's PSUM slot and shouldn't wait for
                            # yb's evacuation too
                            nc.vector.tensor_copy(yT_s[0:64, p, span],
                                                  ya[0:64, :])
                            nc.vector.tensor_copy(
                                den_stack[64 * p:64 * p + 1, :], ya[64:65, :])
                            nc.vector.tensor_copy(yT_s[64:128, p, span],
                                                  yb[0:64, :])
                            nc.vector.tensor_copy(
                                den_stack[64 * p + 32:64 * p + 33, :],
                                yb[64:65, :])
                        if self_norm:
                            # last span: full-width DVE reciprocal per pair
                            # (the custom DVE op wants partition offset 0;
                            # pair-1 rows are memset 1.0 until valid and the
                            # second pass overwrites everything)
                            nc.vector.reciprocal_approx_fast(
                                out=rec32, in_=den_stack)
                            nc.vector.tensor_copy(rec_bf, rec32)
                            if p == 0:
                                # front-insert: the mc0 normalize must fire
                                # early in pair 1's sweep
                                fillers[0:0] = [norm_h(qs, rec_bf, 0),
                                                norm_h(qs, rec_bf, 1)]
                                pace["carry"] += 2.0
                    if not self_norm:
                        # single-instruction DVE reciprocal: no ACT table
                        # switches, no span-boundary exp contention
                        nc.vector.reciprocal_approx_fast(out=rec32,
                                                         in_=den_stack)
                        nc.vector.tensor_copy(rec_bf, rec32)
                    return rec_bf

                def norm_h(qs, rec_bf, h):
                    def f():
                        span = slice(qs * SP, (qs + 1) * SP)
                        mc, r0 = divmod(h, 2)
                        r0 *= 64
                        rb = op_psum.tile([64, SP], f32, tag="op")
                        r0p = 32 * h
                        nc.tensor.matmul(rb,
                                         lhsT=onesb_s[r0p:r0p + 1, 0:64],
                                         rhs=rec_bf[r0p:r0p + 1, :],
                                         start=True, stop=True,
                                         tile_position=(r0p, 0))
                        nc.vector.tensor_mul(yT_s[r0:r0 + 64, mc, span],
                                             yT_s[r0:r0 + 64, mc, span],
                                             rb)
                    return f

                def stage_a2a(qs):
                    def f():
                        span = slice(qs * SP, (qs + 1) * SP)
                        in_r = a2a_in[qs][:].rearrange(
                            "j (two p) q -> two p j q", p=128)
                        for mc in range(2):
                            nc.sync.dma_start(
                                out=in_r[mc],
                                in_=yT_s[:, mc, span].rearrange(
                                    "p (j q) -> p j q", q=QB))
                        nc.gpsimd.collective_compute(
                            "AllToAll", mybir.AluOpType.bypass,
                            replica_groups=ALL8,
                            ins=[a2a_in[qs][:].opt()],
                            outs=[a2a_out[qs][:].opt()])
                    return f

                def proj_qk(w_s, b_s, dst, mc, s):
                    # qT/kT group: out[dims-chunk mc, t-span s]; bias added
                    # on DVE during the PSUM->SBUF evacuation
                    def f():
                        ps = op_psum.tile([128, SP], f32, tag="op",
                                          name="pj")
                        for kc in range(8):
                            nc.tensor.matmul(
                                ps,
                                lhsT=w_s[:, kc, mc * 128:(mc + 1) * 128],
                                rhs=xt_s[:, kc, s * SP:(s + 1) * SP],
                                start=(kc == 0), stop=(kc == 7))
                        nc.vector.tensor_scalar_add(
                            dst[:, mc, s * SP:(s + 1) * SP],
                            ps, b_s[:, mc:mc + 1])
                    return f

                def proj_v(mt):
                    # v tile in natural [t, d] layout; bias via DVE add into
                    # the 65-stride v_aug slots
                    def f():
                        ps = op_psum.tile([128, SP], f32, tag="op",
                                          name="pjv")
                        for kc in range(8):
                            nc.tensor.matmul(
                                ps[:, 0:DL],
                                lhsT=xt_s[:, kc, mt * 128:(mt + 1) * 128],
                                rhs=wv_s[:, kc, :],
                                start=(kc == 0), stop=(kc == 7))
                        nc.vector.tensor_add(
                            v_s[:, mt, :].rearrange(
                                "p (h d) -> p h d", d=65)[:, :, 0:64],
                            ps[:, 0:DL].rearrange("p (h d) -> p h d", d=64),
                            bv_bc.rearrange("p (h d) -> p h d", d=64))
                    return f

                def proj_span_fillers(s):
                    fs = []
                    for mc in range(2):
                        fs.append(proj_qk(wq_s, bq_s, qT_s, mc, s))
                    for mc in range(2):
                        fs.append(proj_qk(wk_s, bk_s, kT_s, mc, s))
                    for mt in range(4 * s, 4 * s + 4):
                        fs.append(proj_v(mt))
                    return fs

                def outproj_pair0_fillers():
                    """Out-projection for spans (0,1) with M=128 (both spans'
                    64-q territories stacked), split into per-(b,ns) filler
                    chunks; spread over span 3's kt loop."""
                    yg = op_sb.tile([128, 8, 2, 2 * QB], bf16, tag="yg",
                                    bufs=1)

                    def gather(sp):
                        def f():
                            out_r = a2a_out[sp][:].rearrange(
                                "(b j2) (h p) q -> b p (j2 h) q", j2=4, p=128)
                            m = sp % 2
                            for b in range(2):
                                nc.sync.dma_start(
                                    out=yg[:, :, b, m * QB:(m + 1) * QB],
                                    in_=out_r[b])
                        return f

                    def po_mms(b, ns, kcs, box, last=False):
                        def f():
                            if not box:
                                box.append(op_psum.tile(
                                    [128, SP], f32, tag="op", name="po"))
                            po = box[0]
                            for kc in kcs:
                                nc.tensor.matmul(
                                    po, lhsT=yg[:, kc, b, :],
                                    rhs=woT_s[:, kc, ns * SP:(ns + 1) * SP],
                                    start=(kc == 0),
                                    stop=(last and kc == kcs[-1]))
                            if last:
                                ob = op_sb.tile([128, SP], f32, tag="ob")
                                nc.vector.tensor_add(
                                    ob, po, bo_bc[:, ns * SP:(ns + 1) * SP])
                                for m in range(2):
                                    nc.gpsimd.dma_start(
                                        out=out_ext[m, b * 64:(b + 1) * 64,
                                                    ns * SP:(ns + 1) * SP],
                                        in_=ob[m * 64:(m + 1) * 64, :])
                        return f

                    fs = [gather(0), gather(1)]
                    for b in range(2):
                        for ns in range(2):
                            box = []
                            fs.append(po_mms(b, ns, [0, 1, 2], box))
                            fs.append(po_mms(b, ns, [3, 4, 5], box))
                            fs.append(po_mms(b, ns, [6, 7], box, last=True))
                    return fs

                def gather_span(sp, tag):
                    yg = op_sb.tile([128, 8, 2, QB], bf16, tag=tag, bufs=1,
                                    name=tag)
                    out_r = a2a_out[sp][:].rearrange(
                        "(b j2) (h p) q -> b p (j2 h) q", j2=4, p=128)
                    for b in range(2):
                        nc.sync.dma_start(out=yg[:, :, b, :], in_=out_r[b])
                    return yg

                def po_unit(sp, yg, b, ns, alt=False):
                    # M=64 out-projection of span sp's territory; the final
                    # units alternate evac/DMA engines so the teardown
                    # doesn't serialize on DVE + one DMA queue
                    def f():
                        po = op_psum.tile([64, SP], f32, tag="op", name="pou")
                        for kc in range(8):
                            nc.tensor.matmul(
                                po, lhsT=yg[:, kc, b, :],
                                rhs=woT_s[:, kc, ns * SP:(ns + 1) * SP],
                                start=(kc == 0), stop=(kc == 7))
                        ob = op_sb.tile([64, SP], f32, tag="ob")
                        nc.vector.tensor_add(
                            ob, po, bo_bc[0:64, ns * SP:(ns + 1) * SP])
                        dq = nc.sync if alt else nc.gpsimd
                        dq.dma_start(
                            out=out_ext[sp, b * 64:(b + 1) * 64,
                                        ns * SP:(ns + 1) * SP],
                            in_=ob)
                    return f

                # prologue: just enough of span 0's projections to start its
                # attention; the rest ride the kt loop as early fillers
                p0 = proj_span_fillers(0)
                for f in [p0[0], p0[2], p0[4], p0[5]]:  # q0, k0, v0, v1
                    f()
                pend = [p0[6], p0[7], p0[1], p0[3]]     # v2, v3, q1, k1
                rec = {}
                for qs in range(QS):
                    if qs < 3:
                        # span qs+1's projections drain during span qs
                        pend.extend(proj_span_fillers(qs + 1))
                    else:
                        # A2A(0)/(1) completed spans ago: pair-0's b=0
                        # out-projection spreads over span 3's kt loop; the
                        # b=1 units are deferred into the A2A(3) flight
                        opf = outproj_pair0_fillers()
                        pend.extend(opf[:8])
                    rec[qs] = attention_span(qs, pend, self_norm=(qs == 3))
                    if qs < 3:
                        nf = [norm_h(qs, rec[qs], h) for h in range(HL)]
                        if qs == 2:
                            # norms/A2A(2) pop first in span 3 so the A2A(2)
                            # flight fully precedes its po consumers
                            pend[0:0] = nf + [stage_a2a(qs)]
                        else:
                            pend.extend(nf + [stage_a2a(qs)])
                for f in pend:
                    f()

                # ---- tail: normalize span-3 mc1, fire the single full-span
                # A2A(3), then fill its ~20us flight with span-2's
                # out-projection, pair-0's deferred b=1 units, and a short
                # dummy-matmul bridge (keeps HAM at full clock so the
                # A2A-gated span-3 out-projection runs warm).
                norm_h(3, rec[3], 2)()
                norm_h(3, rec[3], 3)()
                stage_a2a(3)()
                yg2 = gather_span(2, "yg2")
                for b in range(2):
                    for ns in range(2):
                        po_unit(2, yg2, b, ns)()
                for f in opf[8:]:
                    f()
                dps = sc_psum.tile([128, SP], f32, tag="sc", name="dummy")
                for _ in range(24):
                    nc.tensor.matmul(dps, lhsT=warm_sb[:, 0:128],
                                     rhs=warm_sb, start=True, stop=True)
                yg3 = gather_span(3, "yg3")
                for i, (b, ns) in enumerate(((0, 0), (0, 1), (1, 0), (1, 1))):
                    po_unit(3, yg3, b, ns, alt=(i % 2 == 1))()

    nc.compile()
    return nc


def _get_program():
    if "nc" not in _CACHE:
        _CACHE["nc"] = _build_program()
    return _CACHE["nc"]


def _make_in_maps(x, mask, Wq, bq, Wk, bk, Wv, bv, Wo, bo):
    x = np.asarray(x, np.float32)
    mask = np.asarray(mask, bool)
    Wq = np.asarray(Wq, np.float32)
    Wk = np.asarray(Wk, np.float32)
    Wv = np.asarray(Wv, np.float32)
    Wo = np.asarray(Wo, np.float32)
    bq = np.asarray(bq, np.float32)
    bk = np.asarray(bk, np.float32)
    bv = np.asarray(bv, np.float32)
    bo = np.asarray(bo, np.float32)

    woT = np.ascontiguousarray(Wo.T).astype(BF16)
    in_maps = []
    per_batch = {}
    for b in range(B):
        xTb = np.ascontiguousarray(x[b].T)
        # the only masking the kernel applies is the 128x128 diagonal
        # block (identical for every diagonal tile of a causal mask)
        md = mask[b, 0].T[0:128, 0:128].astype(np.float32)
        per_batch[b] = (xTb, md)
    for c in range(NCORES):
        b, g = divmod(c, GROUPS)
        sl = slice(g * DL, (g + 1) * DL)
        xTb, md = per_batch[b]
        in_maps.append({
            "xT": xTb.astype(BF16),
            "wqT": np.ascontiguousarray((Wq[sl] * SCALE).T).astype(BF16),
            "wkT": np.ascontiguousarray(Wk[sl].T).astype(BF16),
            "wvT": np.ascontiguousarray(Wv[sl].T).astype(BF16),
            "woT": woT,
            "bqP": np.ascontiguousarray((bq[sl] * SCALE).reshape(2, 128).T),
            "bkP": np.ascontiguousarray(bk[sl].reshape(2, 128).T),
            "bv": bv[sl].reshape(1, DL).astype(BF16),
            "bo": bo.reshape(1, D).astype(np.float32),
            "maskd": md.astype(BF16),
            "onesb": np.ones((1, 128), BF16),
        })
    return in_maps


def _capture_profile(nc, in_maps, tmpdir):
    """Run with NTFF capture and process the profile ourselves. Returns
    (results, exec_time_ns|None)."""
    import glob
    import json
    import re
    import subprocess
    from trn_agent_boot.trn_boot import _ntff_profile_via_ctypes
    from concourse import bass2jax

    hook = _ntff_profile_via_ctypes("/opt/axon/libaxon_pjrt.so")
    if hook is None:
        raise RuntimeError("libaxon_pjrt.so lacks NTFF profile symbols")
    os.makedirs(tmpdir, exist_ok=True)
    with hook(tmpdir, [0]):
        results = bass2jax.run_bass_via_pjrt(nc, in_maps, n_cores=NCORES)

    ntffs = glob.glob(os.path.join(tmpdir, "*_body*-device*.ntff"))
    best = None
    for f in ntffs:
        if re.search(r"executable(\d+)-device000000", f):
            if best is None or os.path.getmtime(f) > os.path.getmtime(best):
                best = f
    if best is None:
        raise RuntimeError(f"no NTFF produced in {tmpdir}")
    neff = re.sub(r"-device\d+-execution-\d+\.ntff$", ".neff", best)
    out_json = os.path.join(tmpdir, "prof.json")
    subprocess.check_call(
        ["neuron-profile", "view", "--ignore-nc-buf-usage", "-s", best,
         "-n", neff, "--output-format=json", f"--output-file={out_json}"],
        cwd=tmpdir)
    summary = json.load(open(out_json))["summary"][0]
    return results, int(summary["total_time"] * 1e9)


def kernel(x, mask, Wq, bq, Wk, bk, Wv, bv, Wo, bo):
    from concourse import bass_utils

    in_maps = _make_in_maps(x, mask, Wq, bq, Wk, bk, Wv, bv, Wo, bo)
    nc = _get_program()

    trace = bool(int(os.environ.get("MHA_TRACE", "0")))
    tmpdir = os.environ.get("MHA_TRACE_DIR") or None
    results = None
    if trace and tmpdir:
        try:
            results, exec_ns = _capture_profile(nc, in_maps, tmpdir)
            _CACHE["last_exec_time_ns"] = exec_ns
        except Exception as e:  # profiling is best-effort
            print(f"profiling unavailable: {type(e).__name__}: {e}")
            results = None
    if results is None:
        results = bass_utils.run_bass_kernel_spmd(
            nc, in_maps, core_ids=list(range(NCORES))).results
        _CACHE.setdefault("last_exec_time_ns", None)

    # core c's out[qs] holds rows (q = qs*512 + c*64 + i) for batch 0
    # (rows 0-63) and batch 1 (rows 64-127)
    out = np.empty((B, T, D), np.float32)
    for c in range(NCORES):
        o = results[c]["out"]
        for qs in range(QS):
            q0 = qs * SP + c * QB
            out[0, q0:q0 + QB] = o[qs, 0:QB]
            out[1, q0:q0 + QB] = o[qs, QB:2 * QB]
    return out
